# revision 30
# baseline (speedup 1.0000x reference)
"""Multi-head attention with QK-LayerNorm on 8 TRN2 NeuronCores.

Shapes: B=2, T=2048, E=1024, H=16 heads, S=64 head dim.
Sharding: core c handles batch c//4 and the 4 heads [ (c%4)*4, (c%4)*4+4 ).
Each core computes a partial output (its heads' contribution through Wo);
the host sums the 4 partials per batch and adds bo.

Device-side layout (scores/PV/V/Wo matmuls bf16, Q/K projections fp8e4m3
DoubleRow, f32 PSUM accumulation; host pre-transposes activations and
pre-packs every DRAM tensor into partition-contiguous [128, o, f] layout
so DMAs are single descriptors):
  QT/KT   [s(64)*2heads = 128p, T]  transposed, 2 heads row-packed per tile
  V       [t_k 128p, head, s+1]     extra ones-column -> softmax row sums
  scores  S^T [t_k 128p, t_q 512]   strictly-above-causal blocks skipped
LayerNorm over s (the partition axis of QT) uses matmul statistics
(block-diagonal ones lhsT), f32 row math, then PE "selector" matmuls that
broadcast the per-(head,t) scale/shift rows into PSUM. The ln weight
(w*INV4) is folded into the selector matrix values so the apply is two
tensor_tensor ops (ln bias must be zero, which the input spec guarantees).
Softmax needs no max-subtraction: LN bounds logits to |q.k| <= ~2, so
exp() is taken directly off the scores PSUM (bf16 out), the causal mask
is a 0/1 multiply on diagonal blocks only, and row sums come free from
the ones-column of V. The two head-pair streams are interleaved with the
PV matmul pipelined 3 tiles behind exp so ACT latency never stalls PE.
During attention ACT does exp only: PSUM evictions go to GPSIMD/DVE, the
softmax denominator uses the fast DVE reciprocal, and the Wo matmuls +
output DMA are interleaved per finished 512-query block instead of
running as a serial tail.
"""

import json

import numpy as np
import ml_dtypes

import concourse.bass as bass
import concourse.bass2jax as bass2jax
import concourse.bass_utils as bass_utils
import concourse.tile as tile
from concourse import mybir

B, T, E, H, S = 2, 2048, 1024, 16, 64
HPC = 4            # heads per core
EPC = HPC * S      # feature cols per core = 256
LN_EPS = 1e-5
INV4 = float(E) ** -0.25
FP32 = mybir.dt.float32
BF16 = mybir.dt.bfloat16
FP8 = mybir.dt.float8e4
BF = ml_dtypes.bfloat16
F8 = ml_dtypes.float8_e4m3

# GPSIMD cannot touch PSUM on this toolchain, so PSUM->SBUF evictions go
# to DVE; GPSIMD instead takes half the (SBUF-only) causal mask multiplies.
ATTN_COPY_ENGINE = "vector"

# ---------------------------------------------------------------------------
# Compile hook: this toolchain's walrus accepts at most ONE semaphore wait per
# TPB instruction. Tile attaches several. Split extras into standalone
# EventSemaphore (wait-only) instructions on the same engine.
# ---------------------------------------------------------------------------
_TPB_ENGINES = ("Pool", "Activation", "PE", "DVE", "SP")


def _split_multiwaits(bir_json: bytes) -> bytes:
    d = json.loads(bir_json)
    n_split = 0
    for fn in d.get("functions", []):
        for blk in fn.get("blocks", []):
            insts = blk.get("instructions", [])
            out = []
            for inst in insts:
                si = inst.get("sync_info")
                waits = (si or {}).get("on_wait") or []
                if si and len(waits) > 1 and inst.get("engine") in _TPB_ENGINES:
                    for i, w in enumerate(waits[:-1]):
                        out.append({
                            "debug": inst.get("debug", 0),
                            "engine": inst["engine"],
                            "ins": [],
                            "name": f"{inst['name']}-ws{i}",
                            "opcode": "EventSemaphore",
                            "outs": [],
                            "sync_info": {"on_update": [], "on_wait": [w]},
                        })
                        n_split += 1
                    si["on_wait"] = [waits[-1]]
                out.append(inst)
            blk["instructions"] = out
    return json.dumps(d).encode()


_orig_compile_bir_kernel = bass_utils.compile_bir_kernel


def _patched_compile_bir_kernel(bir_json, tmpdir, neff_name="file.neff"):
    return _orig_compile_bir_kernel(_split_multiwaits(bir_json), tmpdir, neff_name)



bass_utils.compile_bir_kernel = _patched_compile_bir_kernel
bass2jax.compile_bir_kernel = _patched_compile_bir_kernel


def _patched_drain_and_barrier(self, tick_clock, wait_clock):
    # Same as TileContext._drain_and_barrier but the drain's waits are emitted
    # as single-wait instructions (walrus limit).
    gc = tick_clock.global_clock
    ticks = eval(str(gc).replace("VectorClock(", "").rstrip(")"))
    sems = wait_clock.sems.allocated()
    for proc_idx, sem in sems.items():
        t = ticks[proc_idx]
        if t > 0:
            mult = 16 if proc_idx >= 11 else 1
            self.nc.sync.wait_ge(sem, t * mult)
    self.nc.sync.drain()
    self.nc.all_engine_barrier()
    assert self.sems is not None
    popped = self.nc._tile_sem_poison_stack.pop()
    assert popped is self._sem_poison
    self.nc.clear_and_free_semaphores(list(self.sems.allocated().values()))
    self.nc.all_engine_barrier()


tile.TileContext._drain_and_barrier = _patched_drain_and_barrier


# ---------------------------------------------------------------------------
# Device kernel (identical program on all 8 cores)
# ---------------------------------------------------------------------------


def _act_raw(nc, out, in_, func):
    # nc.scalar.activation refuses Reciprocal (accuracy); our tolerance is
    # 2e-2 so the LUT version is fine. Emit InstActivation directly.
    eng = nc.scalar
    inputs = [eng.lower_ap(in_)]
    for arg in (0.0, 1.0, 0.0):  # bias, scale, alpha
        inputs.append(mybir.ImmediateValue(dtype=mybir.dt.float32, value=arg))
    return eng.add_instruction(
        mybir.InstActivation(
            name=nc.get_next_instruction_name(),
            func=func,
            ins=inputs,
            outs=[eng.lower_ap(out)],
        )
    )


def _build_bass():
    nc = bass.Bass()
    xtq_e = nc.dram_tensor("xtq", [128, 8, T], FP8, kind="ExternalInput")
    xtk_e = nc.dram_tensor("xtk", [128, 8, T], FP8, kind="ExternalInput")
    xtv_e = nc.dram_tensor("xtv", [128, 8, T], BF16, kind="ExternalInput")
    wq_e = nc.dram_tensor("wq", [128, 8, EPC], FP8, kind="ExternalInput")
    wk_e = nc.dram_tensor("wk", [128, 8, EPC], FP8, kind="ExternalInput")
    wv_e = nc.dram_tensor("wv", [128, 8, EPC], BF16, kind="ExternalInput")
    wo_e = nc.dram_tensor("wo", [128, 2, E], BF16, kind="ExternalInput")
    masks_e = nc.dram_tensor("masks", [128, 4, 1024], BF16, kind="ExternalInput")
    eye_e = nc.dram_tensor("eye2", [128, 2], BF16, kind="ExternalInput")
    selrep_e = nc.dram_tensor("selrep", [128, 128], BF16, kind="ExternalInput")
    selh_e = nc.dram_tensor("selh", [128, 128], BF16, kind="ExternalInput")
    out_e = nc.dram_tensor("out", [T, E], BF16, kind="ExternalOutput")

    xtq, xtk, xtv = xtq_e.ap(), xtk_e.ap(), xtv_e.ap()
    wq_a, wk_a, wv_a, wo_a = wq_e.ap(), wk_e.ap(), wv_e.ap(), wo_e.ap()
    DR = mybir.MatmulPerfMode.DoubleRow

    with tile.TileContext(nc) as tc:
        with tc.tile_pool(name="singles", bufs=1) as singles, \
             tc.tile_pool(name="xstream", bufs=4) as xstream, \
             tc.tile_pool(name="work", bufs=1) as work, \
             tc.tile_pool(name="rows", bufs=1) as rows, \
             tc.tile_pool(name="expp", bufs=10) as expp, \
             tc.tile_pool(name="outp", bufs=4) as outp, \
             tc.tile_pool(name="otsbp", bufs=2) as otsbp, \
             tc.tile_pool(name="rcp", bufs=6) as rcpp, \
             tc.tile_pool(name="rbp", bufs=2) as rbp, \
             tc.tile_pool(name="psu", bufs=2, space="PSUM") as psu, \
             tc.tile_pool(name="psu1", bufs=4, space="PSUM") as psu1:

            # ---- resident constants (issue order = DMA priority) ---------
            wq_sb = singles.tile([128, 8, EPC], FP8)
            wk_sb = singles.tile([128, 8, EPC], FP8)
            eye_sb = singles.tile([128, 2], BF16)
            nc.scalar.dma_start(out=eye_sb, in_=eye_e.ap())
            selrep_sb = singles.tile([128, 128], BF16)
            nc.scalar.dma_start(out=selrep_sb, in_=selrep_e.ap())
            selh_sb = singles.tile([128, 128], BF16)
            nc.scalar.dma_start(out=selh_sb, in_=selh_e.ap())
            xtv_sb = singles.tile([128, 8, T], BF16)
            wv_sb = singles.tile([128, 8, EPC], BF16)
            masks_sb = singles.tile([128, 4, 1024], BF16)
            wo_sb = singles.tile([128, 2, E], BF16)

            qt = [singles.tile([128, T], BF16, tag=f"qt{m}", name=f"qt{m}") for m in range(2)]
            kt = [singles.tile([128, T], BF16, tag=f"kt{m}", name=f"kt{m}") for m in range(2)]
            vhat = singles.tile([128, 16, HPC, S + 1], BF16)
            otb = [singles.tile([128, T], BF16, tag=f"otb{m}", name=f"otb{m}") for m in range(2)]
            nc.vector.memset(vhat[:, :, :, S:S + 1], 1.0)

            # ---- Q/K projections (fp8 DoubleRow) + LN stats --------------
            sums_t = rows.tile([128, T], FP32)
            sumsq_t = rows.tile([128, T], FP32)

            def ln_stats(src_t, m, c):
                sq = work.tile([128, T], BF16, tag="sq")
                nc.vector.tensor_tensor(out=sq, in0=src_t[m], in1=src_t[m],
                                        op=mybir.AluOpType.mult)
                for n in range(4):
                    sl = slice(n * 512, (n + 1) * 512)
                    ps_s = psu1.tile([128, 512], FP32, tag="u1", name="st_s")
                    ps_q = psu1.tile([128, 512], FP32, tag="u1", name="st_q")
                    nc.tensor.matmul(ps_s[0:2, :], lhsT=eye_sb, rhs=src_t[m][:, sl],
                                     start=True, stop=True)
                    nc.tensor.matmul(ps_q[0:2, :], lhsT=eye_sb, rhs=sq[:, sl],
                                     start=True, stop=True)
                    if n % 2 == 0:
                        nc.scalar.activation(out=sums_t[32 * c:32 * c + 2, sl],
                                             in_=ps_s[0:2, :],
                                             func=mybir.ActivationFunctionType.Copy)
                        nc.scalar.activation(out=sumsq_t[32 * c:32 * c + 2, sl],
                                             in_=ps_q[0:2, :],
                                             func=mybir.ActivationFunctionType.Copy)
                    else:
                        nc.vector.tensor_copy(out=sums_t[32 * c:32 * c + 2, sl],
                                              in_=ps_s[0:2, :])
                        nc.vector.tensor_copy(out=sumsq_t[32 * c:32 * c + 2, sl],
                                              in_=ps_q[0:2, :])

            for qk_i, (x_ap, w_sb, w_a, dst) in enumerate(
                    ((xtq, wq_sb, wq_a, qt), (xtk, wk_sb, wk_a, kt))):
                if qk_i == 1:
                    nc.sync.dma_start(out=wk_sb, in_=wk_a)
                xcs = {}
                for m in range(2):
                    if qk_i == 1 and m == 1:
                        nc.sync.dma_start(out=wv_sb, in_=wv_a)
                        nc.sync.dma_start(out=masks_sb, in_=masks_e.ap())
                        nc.sync.dma_start(out=wo_sb, in_=wo_a)
                    pss = [psu.tile([128, 1024], FP32, tag="u", name=f"pss{j}")
                           for j in range(2)]
                    for kp in range(4):
                        if m == 0:
                            if qk_i == 0:
                                nc.sync.dma_start(out=wq_sb[:, 2 * kp:2 * kp + 2, :],
                                                  in_=wq_a[:, 2 * kp:2 * kp + 2, :])
                            xc = xstream.tile([128, 2, T], FP8, tag="xchunk",
                                              name=f"xc{kp}")
                            nc.sync.dma_start(out=xc, in_=x_ap[:, 2 * kp:2 * kp + 2, :])
                            if qk_i == 1:
                                nc.sync.dma_start(out=xtv_sb[:, 2 * kp:2 * kp + 2, :],
                                                  in_=xtv[:, 2 * kp:2 * kp + 2, :])
                            xcs[kp] = xc
                        xc = xcs[kp]
                        for n in range(4):
                            nc.tensor.matmul(
                                pss[n // 2][:, (n % 2) * 512:(n % 2) * 512 + 512],
                                lhsT=w_sb[:, 2 * kp:2 * kp + 2, m * 128:(m + 1) * 128],
                                rhs=xc[:, :, n * 512:(n + 1) * 512],
                                start=(kp == 0), stop=(kp == 3),
                                perf_mode=DR)
                    for j in range(2):
                        nc.vector.tensor_copy(
                            out=dst[m][:, j * 1024:(j + 1) * 1024], in_=pss[j])
                    ln_stats(dst, m, 2 * qk_i + m)

            # ---- LN row math (overlaps V projection) --------------------
            # mu = sums/S ; var = sumsq/S - mu^2 ; rstd = 1/sqrt(var+eps)
            # a-row = rstd ; c-row = mu*rstd  (w*INV4 folded into selrep)
            eps_col = singles.tile([128, 1], FP32)
            nc.vector.memset(eps_col, LN_EPS)
            tmp = rows.tile([128, T], FP32)
            nc.vector.scalar_tensor_tensor(
                out=tmp, in0=sums_t, scalar=1.0 / S, in1=sums_t,
                op0=mybir.AluOpType.mult, op1=mybir.AluOpType.mult)
            nc.vector.tensor_tensor(out=sumsq_t, in0=sumsq_t, in1=tmp,
                                    op=mybir.AluOpType.subtract)
            nc.scalar.activation(out=sumsq_t, in_=sumsq_t,
                                 func=mybir.ActivationFunctionType.Sqrt,
                                 bias=eps_col, scale=1.0 / S)
            _act_raw(nc, sumsq_t, sumsq_t,
                     mybir.ActivationFunctionType.Reciprocal)             # rstd
            c_bfrows = rows.tile([128, T], BF16)
            a_bfrows = rows.tile([128, T], BF16)
            nc.vector.scalar_tensor_tensor(
                out=c_bfrows, in0=sums_t, scalar=1.0 / S, in1=sumsq_t,
                op0=mybir.AluOpType.mult, op1=mybir.AluOpType.mult)       # mu*rstd
            nc.vector.tensor_copy(out=a_bfrows, in_=sumsq_t)

            # ---- V projection (natural layout + ones column) -------------
            for t16 in range(16):
                psv = psu.tile([128, 1024], FP32, tag="u", name="psv")
                for e8 in range(8):
                    nc.tensor.matmul(
                        psv[:, 0:EPC], lhsT=xtv_sb[:, e8, t16 * 128:(t16 + 1) * 128],
                        rhs=wv_sb[:, e8, :], start=(e8 == 0), stop=(e8 == 7))
                nc.scalar.activation(
                    out=vhat[:, t16, :, 0:S],
                    in_=psv[:, 0:EPC].rearrange("p (h s) -> p h s", h=HPC),
                    func=mybir.ActivationFunctionType.Copy)

            # ---- LN apply via PE row-broadcast --------------------------
            # bpa = (w*INV4)[p] * a_row(f), bpc = (w*INV4)[p] * c_row(f);
            # selrep carries the folded weights so the apply is 2 DVE ops.
            def ln_apply(src_t, m, c):
                sel = selrep_sb[32 * c:32 * c + 2, :]
                for ch in range(4):
                    sl = slice(ch * 512, (ch + 1) * 512)
                    bpa = psu1.tile([128, 512], FP32, tag="u1", name="bpa")
                    bpc = psu1.tile([128, 512], FP32, tag="u1", name="bpc")
                    nc.tensor.matmul(bpa, lhsT=sel,
                                     rhs=a_bfrows[32 * c:32 * c + 2, sl],
                                     start=True, stop=True,
                                     tile_position=(32 * c, 0))
                    nc.tensor.matmul(bpc, lhsT=sel,
                                     rhs=c_bfrows[32 * c:32 * c + 2, sl],
                                     start=True, stop=True,
                                     tile_position=(32 * c, 0))
                    nc.vector.tensor_tensor(out=src_t[m][:, sl], in0=src_t[m][:, sl],
                                            in1=bpa,
                                            op=mybir.AluOpType.mult)
                    nc.vector.tensor_tensor(out=src_t[m][:, sl], in0=src_t[m][:, sl],
                                            in1=bpc,
                                            op=mybir.AluOpType.subtract)

            # ---- attention (two head-pair streams interleaved) -----------
            ln_apply(qt, 0, 0)
            ln_apply(kt, 0, 2)
            ln_apply(qt, 1, 1)
            ln_apply(kt, 1, 3)

            def joint_finish(qb, otsbs):
                # one batched reciprocal for the 4 softmax-sum rows of this
                # query block (both streams x both heads, rows 32-spaced so
                # the PE broadcast matmuls are row-block aligned), then a
                # per-stream PE broadcast + multiply.
                rrec = rbp.tile([128, 512], FP32, tag="rb")
                nc.vector.reciprocal(out=rrec, in_=rcq[qb])
                rbf = rcpp.tile([128, 512], BF16, tag="rcb")
                nc.vector.tensor_copy(out=rbf, in_=rrec)
                for m in range(2):
                    nb = psu.tile([128, 512], FP32, tag="u", name="nb")
                    nc.tensor.matmul(nb, lhsT=selh_sb[64 * m:64 * m + 64, :],
                                     rhs=rbf[64 * m:64 * m + 64, :],
                                     start=True, stop=True,
                                     tile_position=(64 * m, 0))
                    nc.vector.tensor_tensor(
                        out=otb[m][:, qb * 512:(qb + 1) * 512],
                        in0=otsbs[m], in1=nb, op=mybir.AluOpType.mult)

            def emit_wo(t16):
                for e2 in range(2):
                    pso = psu.tile([128, 512], FP32, tag="u", name="pso")
                    for mm in range(2):
                        nc.tensor.matmul(
                            pso,
                            lhsT=otb[mm][:, t16 * 128:(t16 + 1) * 128],
                            rhs=wo_sb[:, mm, e2 * 512:(e2 + 1) * 512],
                            start=(mm == 0), stop=(mm == 1))
                    osb = outp.tile([128, 512], BF16, tag="osb")
                    if e2 == 0:
                        nc.vector.tensor_copy(out=osb, in_=pso)
                    else:
                        nc.scalar.activation(
                            out=osb, in_=pso,
                            func=mybir.ActivationFunctionType.Copy)
                    nc.sync.dma_start(
                        out=out_e.ap()[t16 * 128:(t16 + 1) * 128,
                                       e2 * 512:(e2 + 1) * 512],
                        in_=osb)

            def attn_stream(m):
                for qb in range(4):
                    otps = [psu1.tile([128, 512], FP32, tag="u1",
                                      name=f"otp{m}{h_}") for h_ in range(2)]
                    nkb = 4 * qb + 4
                    exq = []
                    for kb in range(nkb):
                        st = psu.tile([128, 1024], FP32, tag="u", name="st")
                        for h in range(2):
                            pa = slice(64 * h, 64 * h + 64)
                            nc.tensor.matmul(
                                st[:, h * 512:(h + 1) * 512],
                                lhsT=kt[m][pa, kb * 128:(kb + 1) * 128],
                                rhs=qt[m][pa, qb * 512:(qb + 1) * 512],
                                start=True, stop=True)
                        ex = expp.tile([128, 1024], BF16, tag="exp")
                        nc.scalar.activation(
                            out=ex, in_=st,
                            func=mybir.ActivationFunctionType.Exp)
                        d = kb - 4 * qb
                        if d >= 0:  # diagonal block: causal 0/1 mask
                            # early diagonal tiles (pipeline slack) go to the
                            # idle GPSIMD engine; the flush-critical last two
                            # stay on the faster DVE
                            eng = nc.gpsimd if d < 2 else nc.vector
                            eng.tensor_tensor(
                                out=ex, in0=ex, in1=masks_sb[:, d, :],
                                op=mybir.AluOpType.mult)
                        exq.append((ex, kb))
                        if len(exq) > 4:
                            exp_, kb_ = exq.pop(0)
                            for h in range(2):
                                nc.tensor.matmul(
                                    otps[h][0:S + 1, :],
                                    lhsT=vhat[:, kb_, 2 * m + h, :],
                                    rhs=exp_[:, h * 512:(h + 1) * 512],
                                    start=(kb_ == 0), stop=False)
                        yield None
                    while exq:
                        exp_, kb_ = exq.pop(0)
                        for h in range(2):
                            nc.tensor.matmul(
                                otps[h][0:S + 1, :],
                                lhsT=vhat[:, kb_, 2 * m + h, :],
                                rhs=exp_[:, h * 512:(h + 1) * 512],
                                start=(kb_ == 0), stop=(kb_ == nkb - 1))
                    # evict O^T + sums rows off PSUM (DVE); the joint
                    # normalize happens in the driver once both streams
                    # finish this query block.
                    otsb = otsbp.tile([128, 512], FP32, tag="otsb",
                                      name=f"otsb{m}")
                    for h in range(2):
                        p = 64 * m + 32 * h
                        nc.vector.tensor_copy(out=rcq[qb][p:p + 1, :],
                                              in_=otps[h][S:S + 1, :])
                        nc.vector.tensor_copy(out=otsb[64 * h:64 * h + 64, :],
                                              in_=otps[h][0:S, :])
                    yield (qb, otsb)

            # per-qb tiles holding the 4 softmax-sum rows (2 streams x 2
            # heads): (m,h)'s row at partition 64m+32h. Unused partitions
            # are preset to 1.0 so the batched reciprocal stays finite
            # (their selector rows are zero, so they contribute nothing).
            rcq = [rcpp.tile([128, 512], FP32, tag="rc", name=f"rcq{q}")
                   for q in range(4)]
            for q in range(4):
                nc.vector.memset(rcq[q], 1.0)
            g0, g1 = attn_stream(0), attn_stream(1)
            done = [False, False]
            otsb_q = {}
            finished = [0, 0]

            def step(gi, g):
                if done[gi]:
                    return
                try:
                    r = next(g)
                    if isinstance(r, tuple):
                        qb, otsb = r
                        otsb_q[(gi, qb)] = otsb
                        finished[gi] = qb + 1
                        if (1 - gi, qb) in otsb_q:
                            joint_finish(qb, [otsb_q[(0, qb)],
                                              otsb_q[(1, qb)]])
                except StopIteration:
                    done[gi] = True

            for _ in range(6):  # stagger the streams by ~6 kb-steps
                step(0, g0)
            emitted = 0
            while not (done[0] and done[1]):
                step(0, g0)
                step(1, g1)
                while emitted < 4 * min(finished):
                    emit_wo(emitted)
                    emitted += 1
            while emitted < 16:
                emit_wo(emitted)
                emitted += 1
    return nc




_NC_CACHE = None


def _get_nc():
    global _NC_CACHE
    if _NC_CACHE is None:
        _NC_CACHE = _build_bass()
    return _NC_CACHE


# ---------------------------------------------------------------------------
# Host wrapper
# ---------------------------------------------------------------------------

def _make_masks():
    # mask[p, d_idx, f] = 1.0 if p + d <= f else 0, d = 128*d_idx
    p = np.arange(128)[:, None, None]
    dd = (np.arange(4) * 128)[None, :, None]
    f = np.arange(512)[None, None, :]
    m = ((p + dd) <= f).astype(BF)           # [128, 4, 512]
    return np.concatenate([m, m], axis=2)    # [128, 4, 1024] (2 head halves)


def kernel(queries, keys, values, Wq, Wk, Wv, Wo, bo, q_ln_w, q_ln_b,
           k_ln_w, k_ln_b):
    from concourse.bass_utils import run_bass_kernel_spmd

    nc = _get_nc()

    masks = _make_masks()
    eye2 = np.zeros((128, 2), dtype=BF)
    eye2[0:64, 0] = 1
    eye2[64:128, 1] = 1
    # selector rows carry the folded ln weight (w * INV4); ln bias must be 0
    # (guaranteed by the input spec).
    wq_f = np.tile(np.asarray(q_ln_w, np.float32) * INV4, 2)   # [128]
    wk_f = np.tile(np.asarray(k_ln_w, np.float32) * INV4, 2)
    selrep = np.zeros((128, 128), dtype=np.float32)
    for c, wrow in ((0, wq_f), (1, wq_f), (2, wk_f), (3, wk_f)):
        selrep[32 * c, 0:64] = wrow[0:64]
        selrep[32 * c + 1, 64:128] = wrow[64:128]
    selrep = selrep.astype(BF)
    # row 64m+32h = head-h selector for stream m; head h owns otb
    # partitions 64h..64h+64. All other rows stay zero.
    selh = np.zeros((128, 128), dtype=BF)
    for m in range(2):
        selh[64 * m, 0:64] = 1
        selh[64 * m + 32, 64:128] = 1

    in_maps = []
    for core in range(8):
        b = core // 4
        cs = (core % 4) * EPC
        sl = slice(cs, cs + EPC)

        def parr(a, o, dt):
            # [o*128, f] -> [128, o, f] partition-contiguous layout
            a = np.asarray(a, np.float32)
            return np.ascontiguousarray(
                a.reshape(o, 128, a.shape[1]).transpose(1, 0, 2)).astype(dt)
        in_maps.append({
            "xtq": parr(np.asarray(queries[b], np.float32).T, 8, F8),
            "xtk": parr(np.asarray(keys[b], np.float32).T, 8, F8),
            "xtv": parr(np.asarray(values[b], np.float32).T, 8, BF),
            "wq": parr(np.asarray(Wq, np.float32)[:, sl], 8, F8),
            "wk": parr(np.asarray(Wk, np.float32)[:, sl], 8, F8),
            "wv": parr(np.asarray(Wv, np.float32)[:, sl], 8, BF),
            "wo": parr(np.asarray(Wo, np.float32)[sl, :], 2, BF),
            "masks": masks,
            "eye2": eye2,
            "selrep": selrep,
            "selh": selh,
        })

    kernel._last_in_maps = in_maps
    res = run_bass_kernel_spmd(nc, in_maps, core_ids=list(range(8)))
    outs = [res.results[i]["out"].astype(np.float32) for i in range(8)]
    bo32 = np.asarray(bo, np.float32)
    full = np.stack([
        outs[0] + outs[1] + outs[2] + outs[3] + bo32,
        outs[4] + outs[5] + outs[6] + outs[7] + bo32,
    ]).astype(np.float32)
    return full


# revision 33
# speedup vs baseline: 1.1781x; 1.1781x over previous
"""Multi-head attention with QK-LayerNorm on 8 TRN2 NeuronCores.

Shapes: B=2, T=2048, E=1024, H=16 heads, S=64 head dim.
Sharding: core c handles batch c//4 and the 4 heads [ (c%4)*4, (c%4)*4+4 ).
Each core computes a partial output (its heads' contribution through Wo);
the host sums the 4 partials per batch and adds bo.

Device-side layout (scores/PV/V/Wo matmuls bf16, Q/K projections fp8e4m3
DoubleRow, f32 PSUM accumulation; host pre-transposes activations and
pre-packs every DRAM tensor into partition-contiguous [128, o, f] layout
so DMAs are single descriptors):
  QT/KT   [s(64)*2heads = 128p, T]  transposed, 2 heads row-packed per tile
  V       [t_k 128p, head, s+1]     extra ones-column -> softmax row sums
  scores  S^T [t_k 128p, t_q 512]   strictly-above-causal blocks skipped
LayerNorm over s (the partition axis of QT) uses matmul statistics
(block-diagonal ones lhsT), f32 row math, then PE "selector" matmuls that
broadcast the per-(head,t) scale/shift rows into PSUM. The ln weight
(w*INV4) is folded into the selector matrix values so the apply is two
tensor_tensor ops (ln bias must be zero, which the input spec guarantees).
Softmax needs no max-subtraction: LN bounds logits to |q.k| <= ~2, so
exp() is taken directly off the scores PSUM (bf16 out), the causal mask
is a 0/1 multiply on diagonal blocks only, and row sums come free from
the ones-column of V. The two head-pair streams are interleaved with the
PV matmul pipelined 3 tiles behind exp so ACT latency never stalls PE.
During attention ACT does exp only: PSUM evictions go to GPSIMD/DVE, the
softmax denominator uses the fast DVE reciprocal, and the Wo matmuls +
output DMA are interleaved per finished 512-query block instead of
running as a serial tail.
"""

import json

import numpy as np
import ml_dtypes

import concourse.bass as bass
import concourse.bass2jax as bass2jax
import concourse.bass_utils as bass_utils
import concourse.tile as tile
from concourse import mybir

B, T, E, H, S = 2, 2048, 1024, 16, 64
HPC = 4            # heads per core
EPC = HPC * S      # feature cols per core = 256
LN_EPS = 1e-5
INV4 = float(E) ** -0.25
FP32 = mybir.dt.float32
BF16 = mybir.dt.bfloat16
FP8 = mybir.dt.float8e4
BF = ml_dtypes.bfloat16
F8 = ml_dtypes.float8_e4m3

# GPSIMD cannot touch PSUM on this toolchain, so PSUM->SBUF evictions go
# to DVE; GPSIMD instead takes half the (SBUF-only) causal mask multiplies.
ATTN_COPY_ENGINE = "vector"

# ---------------------------------------------------------------------------
# Compile hook: this toolchain's walrus accepts at most ONE semaphore wait per
# TPB instruction. Tile attaches several. Split extras into standalone
# EventSemaphore (wait-only) instructions on the same engine.
# ---------------------------------------------------------------------------
_TPB_ENGINES = ("Pool", "Activation", "PE", "DVE", "SP")


def _split_multiwaits(bir_json: bytes) -> bytes:
    d = json.loads(bir_json)
    n_split = 0
    for fn in d.get("functions", []):
        for blk in fn.get("blocks", []):
            insts = blk.get("instructions", [])
            out = []
            for inst in insts:
                si = inst.get("sync_info")
                waits = (si or {}).get("on_wait") or []
                if si and len(waits) > 1 and inst.get("engine") in _TPB_ENGINES:
                    for i, w in enumerate(waits[:-1]):
                        out.append({
                            "debug": inst.get("debug", 0),
                            "engine": inst["engine"],
                            "ins": [],
                            "name": f"{inst['name']}-ws{i}",
                            "opcode": "EventSemaphore",
                            "outs": [],
                            "sync_info": {"on_update": [], "on_wait": [w]},
                        })
                        n_split += 1
                    si["on_wait"] = [waits[-1]]
                out.append(inst)
            blk["instructions"] = out
    return json.dumps(d).encode()


_orig_compile_bir_kernel = bass_utils.compile_bir_kernel


def _patched_compile_bir_kernel(bir_json, tmpdir, neff_name="file.neff"):
    return _orig_compile_bir_kernel(_split_multiwaits(bir_json), tmpdir, neff_name)



bass_utils.compile_bir_kernel = _patched_compile_bir_kernel
bass2jax.compile_bir_kernel = _patched_compile_bir_kernel


def _patched_drain_and_barrier(self, tick_clock, wait_clock):
    # Same as TileContext._drain_and_barrier but the drain's waits are emitted
    # as single-wait instructions (walrus limit).
    gc = tick_clock.global_clock
    ticks = eval(str(gc).replace("VectorClock(", "").rstrip(")"))
    sems = wait_clock.sems.allocated()
    for proc_idx, sem in sems.items():
        t = ticks[proc_idx]
        if t > 0:
            mult = 16 if proc_idx >= 11 else 1
            self.nc.sync.wait_ge(sem, t * mult)
    self.nc.sync.drain()
    self.nc.all_engine_barrier()
    assert self.sems is not None
    popped = self.nc._tile_sem_poison_stack.pop()
    assert popped is self._sem_poison
    self.nc.clear_and_free_semaphores(list(self.sems.allocated().values()))
    self.nc.all_engine_barrier()


tile.TileContext._drain_and_barrier = _patched_drain_and_barrier


# ---------------------------------------------------------------------------
# Device kernel (identical program on all 8 cores)
# ---------------------------------------------------------------------------


def _act_raw(nc, out, in_, func):
    # nc.scalar.activation refuses Reciprocal (accuracy); our tolerance is
    # 2e-2 so the LUT version is fine. Emit InstActivation directly.
    eng = nc.scalar
    inputs = [eng.lower_ap(in_)]
    for arg in (0.0, 1.0, 0.0):  # bias, scale, alpha
        inputs.append(mybir.ImmediateValue(dtype=mybir.dt.float32, value=arg))
    return eng.add_instruction(
        mybir.InstActivation(
            name=nc.get_next_instruction_name(),
            func=func,
            ins=inputs,
            outs=[eng.lower_ap(out)],
        )
    )


def _build_bass():
    nc = bass.Bass()
    xtq_e = nc.dram_tensor("xtq", [128, 8, T], FP8, kind="ExternalInput")
    xtk_e = nc.dram_tensor("xtk", [128, 8, T], FP8, kind="ExternalInput")
    xtv_e = nc.dram_tensor("xtv", [128, 8, T], BF16, kind="ExternalInput")
    wq_e = nc.dram_tensor("wq", [128, 8, EPC], FP8, kind="ExternalInput")
    wk_e = nc.dram_tensor("wk", [128, 8, EPC], FP8, kind="ExternalInput")
    wv_e = nc.dram_tensor("wv", [128, 8, EPC], BF16, kind="ExternalInput")
    wo_e = nc.dram_tensor("wo", [128, 2, E], BF16, kind="ExternalInput")
    masks_e = nc.dram_tensor("masks", [128, 4, 1024], BF16, kind="ExternalInput")
    eye_e = nc.dram_tensor("eye2", [128, 2], BF16, kind="ExternalInput")
    selrep_e = nc.dram_tensor("selrep", [128, 128], BF16, kind="ExternalInput")
    selh_e = nc.dram_tensor("selh", [128, 128], BF16, kind="ExternalInput")
    out_e = nc.dram_tensor("out", [T, E], BF16, kind="ExternalOutput")

    xtq, xtk, xtv = xtq_e.ap(), xtk_e.ap(), xtv_e.ap()
    wq_a, wk_a, wv_a, wo_a = wq_e.ap(), wk_e.ap(), wv_e.ap(), wo_e.ap()
    DR = mybir.MatmulPerfMode.DoubleRow

    with tile.TileContext(nc) as tc:
        with tc.tile_pool(name="singles", bufs=1) as singles, \
             tc.tile_pool(name="xstream", bufs=4) as xstream, \
             tc.tile_pool(name="work", bufs=1) as work, \
             tc.tile_pool(name="rows", bufs=1) as rows, \
             tc.tile_pool(name="expp", bufs=10) as expp, \
             tc.tile_pool(name="outp", bufs=4) as outp, \
             tc.tile_pool(name="otsbp", bufs=2) as otsbp, \
             tc.tile_pool(name="rcp", bufs=6) as rcpp, \
             tc.tile_pool(name="rbp", bufs=2) as rbp, \
             tc.tile_pool(name="psu", bufs=2, space="PSUM") as psu, \
             tc.tile_pool(name="psu1", bufs=4, space="PSUM") as psu1:

            # ---- resident constants (issue order = DMA priority) ---------
            wq_sb = singles.tile([128, 8, EPC], FP8)
            wk_sb = singles.tile([128, 8, EPC], FP8)
            eye_sb = singles.tile([128, 2], BF16)
            nc.scalar.dma_start(out=eye_sb, in_=eye_e.ap())
            selrep_sb = singles.tile([128, 128], BF16)
            nc.scalar.dma_start(out=selrep_sb, in_=selrep_e.ap())
            selh_sb = singles.tile([128, 128], BF16)
            nc.scalar.dma_start(out=selh_sb, in_=selh_e.ap())
            xtv_sb = singles.tile([128, 8, T], BF16)
            wv_sb = singles.tile([128, 8, EPC], BF16)
            masks_sb = singles.tile([128, 4, 1024], BF16)
            wo_sb = singles.tile([128, 2, E], BF16)

            qt = [singles.tile([128, T], BF16, tag=f"qt{m}", name=f"qt{m}") for m in range(2)]
            kt = [singles.tile([128, T], BF16, tag=f"kt{m}", name=f"kt{m}") for m in range(2)]
            vhat = singles.tile([128, 16, HPC, S + 1], BF16)
            otb = [singles.tile([128, T], BF16, tag=f"otb{m}", name=f"otb{m}") for m in range(2)]
            nc.vector.memset(vhat[:, :, :, S:S + 1], 1.0)

            # ---- Q/K projections (fp8 DoubleRow) + LN stats --------------
            sums_t = rows.tile([128, T], FP32)
            sumsq_t = rows.tile([128, T], FP32)

            def ln_stats(src_t, m, c):
                sq = work.tile([128, T], BF16, tag="sq")
                nc.vector.tensor_tensor(out=sq, in0=src_t[m], in1=src_t[m],
                                        op=mybir.AluOpType.mult)
                for n in range(4):
                    sl = slice(n * 512, (n + 1) * 512)
                    ps_s = psu1.tile([128, 512], FP32, tag="u1", name="st_s")
                    ps_q = psu1.tile([128, 512], FP32, tag="u1", name="st_q")
                    nc.tensor.matmul(ps_s[0:2, :], lhsT=eye_sb, rhs=src_t[m][:, sl],
                                     start=True, stop=True)
                    nc.tensor.matmul(ps_q[0:2, :], lhsT=eye_sb, rhs=sq[:, sl],
                                     start=True, stop=True)
                    if n % 2 == 0:
                        nc.scalar.activation(out=sums_t[32 * c:32 * c + 2, sl],
                                             in_=ps_s[0:2, :],
                                             func=mybir.ActivationFunctionType.Copy)
                        nc.scalar.activation(out=sumsq_t[32 * c:32 * c + 2, sl],
                                             in_=ps_q[0:2, :],
                                             func=mybir.ActivationFunctionType.Copy)
                    else:
                        nc.vector.tensor_copy(out=sums_t[32 * c:32 * c + 2, sl],
                                              in_=ps_s[0:2, :])
                        nc.vector.tensor_copy(out=sumsq_t[32 * c:32 * c + 2, sl],
                                              in_=ps_q[0:2, :])

            for qk_i, (x_ap, w_sb, w_a, dst) in enumerate(
                    ((xtq, wq_sb, wq_a, qt), (xtk, wk_sb, wk_a, kt))):
                if qk_i == 1:
                    nc.sync.dma_start(out=wk_sb, in_=wk_a)
                xcs = {}
                for m in range(2):
                    if qk_i == 1 and m == 1:
                        nc.sync.dma_start(out=wv_sb, in_=wv_a)
                        nc.sync.dma_start(out=masks_sb, in_=masks_e.ap())
                        nc.sync.dma_start(out=wo_sb, in_=wo_a)
                    pss = [psu.tile([128, 1024], FP32, tag="u", name=f"pss{j}")
                           for j in range(2)]
                    for kp in range(4):
                        if m == 0:
                            if qk_i == 0:
                                nc.sync.dma_start(out=wq_sb[:, 2 * kp:2 * kp + 2, :],
                                                  in_=wq_a[:, 2 * kp:2 * kp + 2, :])
                            xc = xstream.tile([128, 2, T], FP8, tag="xchunk",
                                              name=f"xc{kp}")
                            nc.sync.dma_start(out=xc, in_=x_ap[:, 2 * kp:2 * kp + 2, :])
                            if qk_i == 1:
                                nc.sync.dma_start(out=xtv_sb[:, 2 * kp:2 * kp + 2, :],
                                                  in_=xtv[:, 2 * kp:2 * kp + 2, :])
                            xcs[kp] = xc
                        xc = xcs[kp]
                        for n in range(4):
                            nc.tensor.matmul(
                                pss[n // 2][:, (n % 2) * 512:(n % 2) * 512 + 512],
                                lhsT=w_sb[:, 2 * kp:2 * kp + 2, m * 128:(m + 1) * 128],
                                rhs=xc[:, :, n * 512:(n + 1) * 512],
                                start=(kp == 0), stop=(kp == 3),
                                perf_mode=DR)
                    for j in range(2):
                        nc.vector.tensor_copy(
                            out=dst[m][:, j * 1024:(j + 1) * 1024], in_=pss[j])
                    ln_stats(dst, m, 2 * qk_i + m)

            # ---- LN row math (overlaps V projection) --------------------
            # mu = sums/S ; var = sumsq/S - mu^2 ; rstd = 1/sqrt(var+eps)
            # a-row = rstd ; c-row = mu*rstd  (w*INV4 folded into selrep)
            eps_col = singles.tile([128, 1], FP32)
            nc.vector.memset(eps_col, LN_EPS)
            tmp = rows.tile([128, T], FP32)
            nc.vector.scalar_tensor_tensor(
                out=tmp, in0=sums_t, scalar=1.0 / S, in1=sums_t,
                op0=mybir.AluOpType.mult, op1=mybir.AluOpType.mult)
            nc.vector.tensor_tensor(out=sumsq_t, in0=sumsq_t, in1=tmp,
                                    op=mybir.AluOpType.subtract)
            nc.scalar.activation(out=sumsq_t, in_=sumsq_t,
                                 func=mybir.ActivationFunctionType.Sqrt,
                                 bias=eps_col, scale=1.0 / S)
            _act_raw(nc, sumsq_t, sumsq_t,
                     mybir.ActivationFunctionType.Reciprocal)             # rstd
            c_bfrows = rows.tile([128, T], BF16)
            a_bfrows = rows.tile([128, T], BF16)
            nc.vector.scalar_tensor_tensor(
                out=c_bfrows, in0=sums_t, scalar=1.0 / S, in1=sumsq_t,
                op0=mybir.AluOpType.mult, op1=mybir.AluOpType.mult)       # mu*rstd
            nc.vector.tensor_copy(out=a_bfrows, in_=sumsq_t)

            # ---- V projection (natural layout + ones column) -------------
            for t16 in range(16):
                psv = psu.tile([128, 1024], FP32, tag="u", name="psv")
                for e8 in range(8):
                    nc.tensor.matmul(
                        psv[:, 0:EPC], lhsT=xtv_sb[:, e8, t16 * 128:(t16 + 1) * 128],
                        rhs=wv_sb[:, e8, :], start=(e8 == 0), stop=(e8 == 7))
                nc.scalar.activation(
                    out=vhat[:, t16, :, 0:S],
                    in_=psv[:, 0:EPC].rearrange("p (h s) -> p h s", h=HPC),
                    func=mybir.ActivationFunctionType.Copy)

            # ---- LN apply via PE row-broadcast --------------------------
            # bpa = (w*INV4)[p] * a_row(f), bpc = (w*INV4)[p] * c_row(f);
            # selrep carries the folded weights so the apply is 2 DVE ops.
            def ln_apply(src_t, m, c):
                sel = selrep_sb[32 * c:32 * c + 2, :]
                for ch in range(4):
                    sl = slice(ch * 512, (ch + 1) * 512)
                    bpa = psu1.tile([128, 512], FP32, tag="u1", name="bpa")
                    bpc = psu1.tile([128, 512], FP32, tag="u1", name="bpc")
                    nc.tensor.matmul(bpa, lhsT=sel,
                                     rhs=a_bfrows[32 * c:32 * c + 2, sl],
                                     start=True, stop=True,
                                     tile_position=(32 * c, 0))
                    nc.tensor.matmul(bpc, lhsT=sel,
                                     rhs=c_bfrows[32 * c:32 * c + 2, sl],
                                     start=True, stop=True,
                                     tile_position=(32 * c, 0))
                    nc.vector.tensor_tensor(out=src_t[m][:, sl], in0=src_t[m][:, sl],
                                            in1=bpa,
                                            op=mybir.AluOpType.mult)
                    nc.vector.tensor_tensor(out=src_t[m][:, sl], in0=src_t[m][:, sl],
                                            in1=bpc,
                                            op=mybir.AluOpType.subtract)

            # ---- attention (two head-pair streams interleaved) -----------
            ln_apply(qt, 0, 0)
            ln_apply(kt, 0, 2)
            ln_apply(qt, 1, 1)
            ln_apply(kt, 1, 3)

            def joint_finish(qb, otsbs):
                # one batched reciprocal for the 4 softmax-sum rows of this
                # query block (both streams x both heads, rows 32-spaced so
                # the PE broadcast matmuls are row-block aligned), then a
                # per-stream PE broadcast + multiply.
                rrec = rbp.tile([128, 512], FP32, tag="rb")
                nc.vector.reciprocal(out=rrec, in_=rcq[qb])
                rbf = rcpp.tile([128, 512], BF16, tag="rcb")
                nc.vector.tensor_copy(out=rbf, in_=rrec)
                for m in range(2):
                    nb = psu.tile([128, 512], FP32, tag="u", name="nb")
                    nc.tensor.matmul(nb, lhsT=selh_sb[64 * m:64 * m + 64, :],
                                     rhs=rbf[64 * m:64 * m + 64, :],
                                     start=True, stop=True,
                                     tile_position=(64 * m, 0))
                    nc.vector.tensor_tensor(
                        out=otb[m][:, qb * 512:(qb + 1) * 512],
                        in0=otsbs[m], in1=nb, op=mybir.AluOpType.mult)

            def emit_wo(t16):
                for e2 in range(2):
                    pso = psu.tile([128, 512], FP32, tag="u", name="pso")
                    for mm in range(2):
                        nc.tensor.matmul(
                            pso,
                            lhsT=otb[mm][:, t16 * 128:(t16 + 1) * 128],
                            rhs=wo_sb[:, mm, e2 * 512:(e2 + 1) * 512],
                            start=(mm == 0), stop=(mm == 1))
                    osb = outp.tile([128, 512], BF16, tag="osb")
                    nc.vector.tensor_copy(out=osb, in_=pso)
                    nc.sync.dma_start(
                        out=out_e.ap()[t16 * 128:(t16 + 1) * 128,
                                       e2 * 512:(e2 + 1) * 512],
                        in_=osb)

            def attn_stream(m):
                for qb in range(4):
                    otps = [psu1.tile([128, 512], FP32, tag="u1",
                                      name=f"otp{m}{h_}") for h_ in range(2)]
                    nkb = 4 * qb + 4
                    exq = []
                    for kb in range(nkb):
                        st = psu.tile([128, 1024], FP32, tag="u", name="st")
                        for h in range(2):
                            pa = slice(64 * h, 64 * h + 64)
                            nc.tensor.matmul(
                                st[:, h * 512:(h + 1) * 512],
                                lhsT=kt[m][pa, kb * 128:(kb + 1) * 128],
                                rhs=qt[m][pa, qb * 512:(qb + 1) * 512],
                                start=True, stop=True)
                        ex = expp.tile([128, 1024], BF16, tag="exp")
                        nc.scalar.activation(
                            out=ex, in_=st,
                            func=mybir.ActivationFunctionType.Exp)
                        d = kb - 4 * qb
                        if d >= 0:  # diagonal block: causal 0/1 mask
                            # early diagonal tiles (pipeline slack) go to the
                            # idle GPSIMD engine; the flush-critical last two
                            # stay on the faster DVE
                            eng = nc.gpsimd if d < 2 else nc.vector
                            eng.tensor_tensor(
                                out=ex, in0=ex, in1=masks_sb[:, d, :],
                                op=mybir.AluOpType.mult)
                        exq.append((ex, kb))
                        if len(exq) > 3:
                            exp_, kb_ = exq.pop(0)
                            for h in range(2):
                                nc.tensor.matmul(
                                    otps[h][0:S + 1, :],
                                    lhsT=vhat[:, kb_, 2 * m + h, :],
                                    rhs=exp_[:, h * 512:(h + 1) * 512],
                                    start=(kb_ == 0), stop=False)
                        yield None
                    while exq:
                        exp_, kb_ = exq.pop(0)
                        for h in range(2):
                            nc.tensor.matmul(
                                otps[h][0:S + 1, :],
                                lhsT=vhat[:, kb_, 2 * m + h, :],
                                rhs=exp_[:, h * 512:(h + 1) * 512],
                                start=(kb_ == 0), stop=(kb_ == nkb - 1))
                    # evict O^T + sums rows off PSUM (DVE); the joint
                    # normalize happens in the driver once both streams
                    # finish this query block.
                    otsb = otsbp.tile([128, 512], FP32, tag="otsb",
                                      name=f"otsb{m}")
                    for h in range(2):
                        p = 64 * m + 32 * h
                        nc.vector.tensor_copy(out=rcq[qb][p:p + 1, :],
                                              in_=otps[h][S:S + 1, :])
                        nc.vector.tensor_copy(out=otsb[64 * h:64 * h + 64, :],
                                              in_=otps[h][0:S, :])
                    yield (qb, otsb)

            # per-qb tiles holding the 4 softmax-sum rows (2 streams x 2
            # heads): (m,h)'s row at partition 64m+32h. Unused partitions
            # are preset to 1.0 so the batched reciprocal stays finite
            # (their selector rows are zero, so they contribute nothing).
            rcq = [rcpp.tile([128, 512], FP32, tag="rc", name=f"rcq{q}")
                   for q in range(4)]
            for q in range(4):
                nc.vector.memset(rcq[q], 1.0)
            g0, g1 = attn_stream(0), attn_stream(1)
            done = [False, False]
            otsb_q = {}
            finished = [0, 0]

            def step(gi, g):
                if done[gi]:
                    return
                try:
                    r = next(g)
                    if isinstance(r, tuple):
                        qb, otsb = r
                        otsb_q[(gi, qb)] = otsb
                        finished[gi] = qb + 1
                        if (1 - gi, qb) in otsb_q:
                            joint_finish(qb, [otsb_q[(0, qb)],
                                              otsb_q[(1, qb)]])
                except StopIteration:
                    done[gi] = True

            for _ in range(2):  # stagger the streams by 2 kb-steps
                step(0, g0)
            emitted = 0
            while not (done[0] and done[1]):
                step(0, g0)
                step(1, g1)
                while emitted < 4 * min(finished):
                    emit_wo(emitted)
                    emitted += 1
            while emitted < 16:
                emit_wo(emitted)
                emitted += 1
    return nc




_NC_CACHE = None


def _get_nc():
    global _NC_CACHE
    if _NC_CACHE is None:
        _NC_CACHE = _build_bass()
    return _NC_CACHE


# ---------------------------------------------------------------------------
# Host wrapper
# ---------------------------------------------------------------------------

def _make_masks():
    # mask[p, d_idx, f] = 1.0 if p + d <= f else 0, d = 128*d_idx
    p = np.arange(128)[:, None, None]
    dd = (np.arange(4) * 128)[None, :, None]
    f = np.arange(512)[None, None, :]
    m = ((p + dd) <= f).astype(BF)           # [128, 4, 512]
    return np.concatenate([m, m], axis=2)    # [128, 4, 1024] (2 head halves)


def kernel(queries, keys, values, Wq, Wk, Wv, Wo, bo, q_ln_w, q_ln_b,
           k_ln_w, k_ln_b):
    from concourse.bass_utils import run_bass_kernel_spmd

    nc = _get_nc()

    masks = _make_masks()
    eye2 = np.zeros((128, 2), dtype=BF)
    eye2[0:64, 0] = 1
    eye2[64:128, 1] = 1
    # selector rows carry the folded ln weight (w * INV4); ln bias must be 0
    # (guaranteed by the input spec).
    wq_f = np.tile(np.asarray(q_ln_w, np.float32) * INV4, 2)   # [128]
    wk_f = np.tile(np.asarray(k_ln_w, np.float32) * INV4, 2)
    selrep = np.zeros((128, 128), dtype=np.float32)
    for c, wrow in ((0, wq_f), (1, wq_f), (2, wk_f), (3, wk_f)):
        selrep[32 * c, 0:64] = wrow[0:64]
        selrep[32 * c + 1, 64:128] = wrow[64:128]
    selrep = selrep.astype(BF)
    # row 64m+32h = head-h selector for stream m; head h owns otb
    # partitions 64h..64h+64. All other rows stay zero.
    selh = np.zeros((128, 128), dtype=BF)
    for m in range(2):
        selh[64 * m, 0:64] = 1
        selh[64 * m + 32, 64:128] = 1

    in_maps = []
    for core in range(8):
        b = core // 4
        cs = (core % 4) * EPC
        sl = slice(cs, cs + EPC)

        def parr(a, o, dt):
            # [o*128, f] -> [128, o, f] partition-contiguous layout
            a = np.asarray(a, np.float32)
            return np.ascontiguousarray(
                a.reshape(o, 128, a.shape[1]).transpose(1, 0, 2)).astype(dt)
        in_maps.append({
            "xtq": parr(np.asarray(queries[b], np.float32).T, 8, F8),
            "xtk": parr(np.asarray(keys[b], np.float32).T, 8, F8),
            "xtv": parr(np.asarray(values[b], np.float32).T, 8, BF),
            "wq": parr(np.asarray(Wq, np.float32)[:, sl], 8, F8),
            "wk": parr(np.asarray(Wk, np.float32)[:, sl], 8, F8),
            "wv": parr(np.asarray(Wv, np.float32)[:, sl], 8, BF),
            "wo": parr(np.asarray(Wo, np.float32)[sl, :], 2, BF),
            "masks": masks,
            "eye2": eye2,
            "selrep": selrep,
            "selh": selh,
        })

    kernel._last_in_maps = in_maps
    res = run_bass_kernel_spmd(nc, in_maps, core_ids=list(range(8)))
    outs = [res.results[i]["out"].astype(np.float32) for i in range(8)]
    bo32 = np.asarray(bo, np.float32)
    full = np.stack([
        outs[0] + outs[1] + outs[2] + outs[3] + bo32,
        outs[4] + outs[5] + outs[6] + outs[7] + bo32,
    ]).astype(np.float32)
    return full


# revision 42
# speedup vs baseline: 1.1792x; 1.0009x over previous
"""Multi-head attention with QK-LayerNorm on 8 TRN2 NeuronCores.

Shapes: B=2, T=2048, E=1024, H=16 heads, S=64 head dim.
Sharding: core c handles batch c//4 and the 4 heads [ (c%4)*4, (c%4)*4+4 ).
Each core computes a partial output (its heads' contribution through Wo);
the host sums the 4 partials per batch and adds bo.

Device-side layout (scores/PV/V/Wo matmuls bf16, Q/K projections fp8e4m3
DoubleRow, f32 PSUM accumulation; host pre-transposes activations and
pre-packs every DRAM tensor into partition-contiguous [128, o, f] layout
so DMAs are single descriptors):
  QT/KT   [s(64)*2heads = 128p, T]  transposed, 2 heads row-packed per tile
  V       [t_k 128p, head, s+1]     extra ones-column -> softmax row sums
  scores  S^T [t_k 128p, t_q 512]   strictly-above-causal blocks skipped
LayerNorm over s (the partition axis of QT) uses matmul statistics
(block-diagonal ones lhsT), f32 row math, then PE "selector" matmuls that
broadcast the per-(head,t) scale/shift rows into PSUM. The ln weight
(w*INV4) is folded into the selector matrix values so the apply is two
tensor_tensor ops (ln bias must be zero, which the input spec guarantees).
Softmax needs no max-subtraction: LN bounds logits to |q.k| <= ~2, so
exp() is taken directly off the scores PSUM (bf16 out), the causal mask
is a 0/1 multiply on diagonal blocks only, and row sums come free from
the ones-column of V. The two head-pair streams are interleaved with the
PV matmul pipelined 3 tiles behind exp so ACT latency never stalls PE.
During attention ACT does exp only: PSUM evictions go to GPSIMD/DVE, the
softmax denominator uses the fast DVE reciprocal, and the Wo matmuls +
output DMA are interleaved per finished 512-query block instead of
running as a serial tail.
"""

import json

import numpy as np
import ml_dtypes

import concourse.bass as bass
import concourse.bass2jax as bass2jax
import concourse.bass_utils as bass_utils
import concourse.tile as tile
from concourse import mybir

B, T, E, H, S = 2, 2048, 1024, 16, 64
HPC = 4            # heads per core
EPC = HPC * S      # feature cols per core = 256
LN_EPS = 1e-5
INV4 = float(E) ** -0.25
FP32 = mybir.dt.float32
BF16 = mybir.dt.bfloat16
FP8 = mybir.dt.float8e4
BF = ml_dtypes.bfloat16
F8 = ml_dtypes.float8_e4m3

# GPSIMD cannot touch PSUM on this toolchain, so PSUM->SBUF evictions go
# to DVE; GPSIMD instead takes half the (SBUF-only) causal mask multiplies.
ATTN_COPY_ENGINE = "vector"

# ---------------------------------------------------------------------------
# Compile hook: this toolchain's walrus accepts at most ONE semaphore wait per
# TPB instruction. Tile attaches several. Split extras into standalone
# EventSemaphore (wait-only) instructions on the same engine.
# ---------------------------------------------------------------------------
_TPB_ENGINES = ("Pool", "Activation", "PE", "DVE", "SP")


def _split_multiwaits(bir_json: bytes) -> bytes:
    d = json.loads(bir_json)
    n_split = 0
    for fn in d.get("functions", []):
        for blk in fn.get("blocks", []):
            insts = blk.get("instructions", [])
            out = []
            for inst in insts:
                si = inst.get("sync_info")
                waits = (si or {}).get("on_wait") or []
                if si and len(waits) > 1 and inst.get("engine") in _TPB_ENGINES:
                    for i, w in enumerate(waits[:-1]):
                        out.append({
                            "debug": inst.get("debug", 0),
                            "engine": inst["engine"],
                            "ins": [],
                            "name": f"{inst['name']}-ws{i}",
                            "opcode": "EventSemaphore",
                            "outs": [],
                            "sync_info": {"on_update": [], "on_wait": [w]},
                        })
                        n_split += 1
                    si["on_wait"] = [waits[-1]]
                out.append(inst)
            blk["instructions"] = out
    return json.dumps(d).encode()


_orig_compile_bir_kernel = bass_utils.compile_bir_kernel


def _patched_compile_bir_kernel(bir_json, tmpdir, neff_name="file.neff"):
    return _orig_compile_bir_kernel(_split_multiwaits(bir_json), tmpdir, neff_name)



bass_utils.compile_bir_kernel = _patched_compile_bir_kernel
bass2jax.compile_bir_kernel = _patched_compile_bir_kernel


def _patched_drain_and_barrier(self, tick_clock, wait_clock):
    # Same as TileContext._drain_and_barrier but the drain's waits are emitted
    # as single-wait instructions (walrus limit).
    gc = tick_clock.global_clock
    ticks = eval(str(gc).replace("VectorClock(", "").rstrip(")"))
    sems = wait_clock.sems.allocated()
    for proc_idx, sem in sems.items():
        t = ticks[proc_idx]
        if t > 0:
            mult = 16 if proc_idx >= 11 else 1
            self.nc.sync.wait_ge(sem, t * mult)
    self.nc.sync.drain()
    self.nc.all_engine_barrier()
    assert self.sems is not None
    popped = self.nc._tile_sem_poison_stack.pop()
    assert popped is self._sem_poison
    self.nc.clear_and_free_semaphores(list(self.sems.allocated().values()))
    self.nc.all_engine_barrier()


tile.TileContext._drain_and_barrier = _patched_drain_and_barrier


# ---------------------------------------------------------------------------
# Device kernel (identical program on all 8 cores)
# ---------------------------------------------------------------------------


def _act_raw(nc, out, in_, func):
    # nc.scalar.activation refuses Reciprocal (accuracy); our tolerance is
    # 2e-2 so the LUT version is fine. Emit InstActivation directly.
    eng = nc.scalar
    inputs = [eng.lower_ap(in_)]
    for arg in (0.0, 1.0, 0.0):  # bias, scale, alpha
        inputs.append(mybir.ImmediateValue(dtype=mybir.dt.float32, value=arg))
    return eng.add_instruction(
        mybir.InstActivation(
            name=nc.get_next_instruction_name(),
            func=func,
            ins=inputs,
            outs=[eng.lower_ap(out)],
        )
    )


def _build_bass():
    nc = bass.Bass()
    xtq_e = nc.dram_tensor("xtq", [128, 8, T], FP8, kind="ExternalInput")
    xtk_e = nc.dram_tensor("xtk", [128, 8, T], FP8, kind="ExternalInput")
    xtv_e = nc.dram_tensor("xtv", [128, 8, T], BF16, kind="ExternalInput")
    wq_e = nc.dram_tensor("wq", [128, 8, EPC], FP8, kind="ExternalInput")
    wk_e = nc.dram_tensor("wk", [128, 8, EPC], FP8, kind="ExternalInput")
    wv_e = nc.dram_tensor("wv", [128, 8, EPC], BF16, kind="ExternalInput")
    wo_e = nc.dram_tensor("wo", [128, 2, E], BF16, kind="ExternalInput")
    masks_e = nc.dram_tensor("masks", [128, 4, 1024], BF16, kind="ExternalInput")
    eye_e = nc.dram_tensor("eye2", [128, 2], BF16, kind="ExternalInput")
    selrep_e = nc.dram_tensor("selrep", [128, 4, 128], BF16, kind="ExternalInput")
    selh_e = nc.dram_tensor("selh", [128, 2, 128], BF16, kind="ExternalInput")
    out_e = nc.dram_tensor("out", [T, E], BF16, kind="ExternalOutput")

    xtq, xtk, xtv = xtq_e.ap(), xtk_e.ap(), xtv_e.ap()
    wq_a, wk_a, wv_a, wo_a = wq_e.ap(), wk_e.ap(), wv_e.ap(), wo_e.ap()
    DR = mybir.MatmulPerfMode.DoubleRow

    with tile.TileContext(nc) as tc:
        with tc.tile_pool(name="singles", bufs=1) as singles, \
             tc.tile_pool(name="xstream", bufs=4) as xstream, \
             tc.tile_pool(name="work", bufs=1) as work, \
             tc.tile_pool(name="rows", bufs=1) as rows, \
             tc.tile_pool(name="expp", bufs=10) as expp, \
             tc.tile_pool(name="outp", bufs=4) as outp, \
             tc.tile_pool(name="otsbp", bufs=2) as otsbp, \
             tc.tile_pool(name="rcp", bufs=6) as rcpp, \
             tc.tile_pool(name="rbp", bufs=2) as rbp, \
             tc.tile_pool(name="psu", bufs=2, space="PSUM") as psu, \
             tc.tile_pool(name="psu1", bufs=4, space="PSUM") as psu1:

            # ---- resident constants (issue order = DMA priority) ---------
            wq_sb = singles.tile([128, 8, EPC], FP8)
            wk_sb = singles.tile([128, 8, EPC], FP8)
            eye_sb = singles.tile([128, 2], BF16)
            nc.scalar.dma_start(out=eye_sb, in_=eye_e.ap())
            selrep_sb = singles.tile([128, 4, 128], BF16)
            nc.scalar.dma_start(out=selrep_sb, in_=selrep_e.ap())
            selh_sb = singles.tile([128, 2, 128], BF16)
            nc.scalar.dma_start(out=selh_sb, in_=selh_e.ap())
            xtv_sb = singles.tile([128, 8, T], BF16)
            wv_sb = singles.tile([128, 8, EPC], BF16)
            masks_sb = singles.tile([128, 4, 1024], BF16)
            wo_sb = singles.tile([128, 2, E], BF16)

            qt = [singles.tile([128, T], BF16, tag=f"qt{m}", name=f"qt{m}") for m in range(2)]
            kt = [singles.tile([128, T], BF16, tag=f"kt{m}", name=f"kt{m}") for m in range(2)]
            # per-head K tiles, zero-padded in the other head's rows so the
            # score matmuls contract over the full 128 partitions (K=128
            # matmuls run ~1.8x faster per column than K=64 on this HW)
            kth = [[singles.tile([128, T], BF16, tag=f"kth{m}{h}",
                                 name=f"kth{m}{h}") for h in range(2)]
                   for m in range(2)]
            vhat = singles.tile([128, 16, HPC, S + 1], BF16)
            otb = [singles.tile([128, T], BF16, tag=f"otb{m}", name=f"otb{m}") for m in range(2)]
            nc.vector.memset(vhat[:, :, :, S:S + 1], 1.0)
            for m in range(2):
                nc.vector.memset(kth[m][0][64:128, :], 0.0)
                nc.vector.memset(kth[m][1][0:64, :], 0.0)

            # ---- Q/K projections (fp8 DoubleRow) + LN stats --------------
            # stat rows live at partitions {32c, 32c+1}; the rest are preset
            # finite so the K=128-padded selector matmuls stay NaN-free
            sums_t = rows.tile([128, T], FP32)
            sumsq_t = rows.tile([128, T], FP32)
            nc.vector.memset(sums_t, 1.0)
            nc.vector.memset(sumsq_t, 1.0)

            def ln_stats(src_t, m, c):
                sq = work.tile([128, T], BF16, tag="sq")
                nc.vector.tensor_tensor(out=sq, in0=src_t[m], in1=src_t[m],
                                        op=mybir.AluOpType.mult)
                for n in range(4):
                    sl = slice(n * 512, (n + 1) * 512)
                    ps_s = psu1.tile([128, 512], FP32, tag="u1", name="st_s")
                    ps_q = psu1.tile([128, 512], FP32, tag="u1", name="st_q")
                    nc.tensor.matmul(ps_s[0:2, :], lhsT=eye_sb, rhs=src_t[m][:, sl],
                                     start=True, stop=True)
                    nc.tensor.matmul(ps_q[0:2, :], lhsT=eye_sb, rhs=sq[:, sl],
                                     start=True, stop=True)
                    if n % 2 == 0:
                        nc.scalar.activation(out=sums_t[32 * c:32 * c + 2, sl],
                                             in_=ps_s[0:2, :],
                                             func=mybir.ActivationFunctionType.Copy)
                        nc.scalar.activation(out=sumsq_t[32 * c:32 * c + 2, sl],
                                             in_=ps_q[0:2, :],
                                             func=mybir.ActivationFunctionType.Copy)
                    else:
                        nc.vector.tensor_copy(out=sums_t[32 * c:32 * c + 2, sl],
                                              in_=ps_s[0:2, :])
                        nc.vector.tensor_copy(out=sumsq_t[32 * c:32 * c + 2, sl],
                                              in_=ps_q[0:2, :])

            for qk_i, (x_ap, w_sb, w_a, dst) in enumerate(
                    ((xtq, wq_sb, wq_a, qt), (xtk, wk_sb, wk_a, kt))):
                if qk_i == 1:
                    nc.sync.dma_start(out=wk_sb, in_=wk_a)
                xcs = {}
                for m in range(2):
                    if qk_i == 1 and m == 1:
                        nc.sync.dma_start(out=wv_sb, in_=wv_a)
                        nc.sync.dma_start(out=masks_sb, in_=masks_e.ap())
                        nc.sync.dma_start(out=wo_sb, in_=wo_a)
                    pss = [psu.tile([128, 1024], FP32, tag="u", name=f"pss{j}")
                           for j in range(2)]
                    for kp in range(4):
                        if m == 0:
                            if qk_i == 0:
                                nc.sync.dma_start(out=wq_sb[:, 2 * kp:2 * kp + 2, :],
                                                  in_=wq_a[:, 2 * kp:2 * kp + 2, :])
                            xc = xstream.tile([128, 2, T], FP8, tag="xchunk",
                                              name=f"xc{kp}")
                            nc.sync.dma_start(out=xc, in_=x_ap[:, 2 * kp:2 * kp + 2, :])
                            if qk_i == 1:
                                nc.sync.dma_start(out=xtv_sb[:, 2 * kp:2 * kp + 2, :],
                                                  in_=xtv[:, 2 * kp:2 * kp + 2, :])
                            xcs[kp] = xc
                        xc = xcs[kp]
                        for n in range(4):
                            nc.tensor.matmul(
                                pss[n // 2][:, (n % 2) * 512:(n % 2) * 512 + 512],
                                lhsT=w_sb[:, 2 * kp:2 * kp + 2, m * 128:(m + 1) * 128],
                                rhs=xc[:, :, n * 512:(n + 1) * 512],
                                start=(kp == 0), stop=(kp == 3),
                                perf_mode=DR)
                    for j in range(2):
                        nc.vector.tensor_copy(
                            out=dst[m][:, j * 1024:(j + 1) * 1024], in_=pss[j])
                    ln_stats(dst, m, 2 * qk_i + m)

            # ---- LN row math (overlaps V projection) --------------------
            # mu = sums/S ; var = sumsq/S - mu^2 ; rstd = 1/sqrt(var+eps)
            # a-row = rstd ; c-row = mu*rstd  (w*INV4 folded into selrep)
            eps_col = singles.tile([128, 1], FP32)
            nc.vector.memset(eps_col, LN_EPS)
            tmp = rows.tile([128, T], FP32)
            nc.vector.scalar_tensor_tensor(
                out=tmp, in0=sums_t, scalar=1.0 / S, in1=sums_t,
                op0=mybir.AluOpType.mult, op1=mybir.AluOpType.mult)
            nc.vector.tensor_tensor(out=sumsq_t, in0=sumsq_t, in1=tmp,
                                    op=mybir.AluOpType.subtract)
            nc.scalar.activation(out=sumsq_t, in_=sumsq_t,
                                 func=mybir.ActivationFunctionType.Sqrt,
                                 bias=eps_col, scale=1.0 / S)
            _act_raw(nc, sumsq_t, sumsq_t,
                     mybir.ActivationFunctionType.Reciprocal)             # rstd
            c_bfrows = rows.tile([128, T], BF16)
            a_bfrows = rows.tile([128, T], BF16)
            nc.vector.scalar_tensor_tensor(
                out=c_bfrows, in0=sums_t, scalar=1.0 / S, in1=sumsq_t,
                op0=mybir.AluOpType.mult, op1=mybir.AluOpType.mult)       # mu*rstd
            nc.vector.tensor_copy(out=a_bfrows, in_=sumsq_t)

            # ---- V projection (natural layout + ones column) -------------
            for t16 in range(16):
                psv = psu.tile([128, 1024], FP32, tag="u", name="psv")
                for e8 in range(8):
                    nc.tensor.matmul(
                        psv[:, 0:EPC], lhsT=xtv_sb[:, e8, t16 * 128:(t16 + 1) * 128],
                        rhs=wv_sb[:, e8, :], start=(e8 == 0), stop=(e8 == 7))
                nc.scalar.activation(
                    out=vhat[:, t16, :, 0:S],
                    in_=psv[:, 0:EPC].rearrange("p (h s) -> p h s", h=HPC),
                    func=mybir.ActivationFunctionType.Copy)

            # ---- LN apply via PE row-broadcast --------------------------
            # bpa = (w*INV4)[p] * a_row(f), bpc = (w*INV4)[p] * c_row(f);
            # selrep carries the folded weights so the apply is 2 DVE ops.
            # K padded to the full 128 partitions (unused selector rows are
            # zero, the row tiles are finite everywhere).
            def ln_apply(src_t, m, c, split_out=None):
                sel = selrep_sb[:, c, :]
                for ch in range(4):
                    sl = slice(ch * 512, (ch + 1) * 512)
                    bpa = psu1.tile([128, 512], FP32, tag="u1", name="bpa")
                    bpc = psu1.tile([128, 512], FP32, tag="u1", name="bpc")
                    nc.tensor.matmul(bpa, lhsT=sel, rhs=a_bfrows[:, sl],
                                     start=True, stop=True)
                    nc.tensor.matmul(bpc, lhsT=sel, rhs=c_bfrows[:, sl],
                                     start=True, stop=True)
                    nc.vector.tensor_tensor(out=src_t[m][:, sl], in0=src_t[m][:, sl],
                                            in1=bpa,
                                            op=mybir.AluOpType.mult)
                    if split_out is None:
                        nc.vector.tensor_tensor(out=src_t[m][:, sl],
                                                in0=src_t[m][:, sl], in1=bpc,
                                                op=mybir.AluOpType.subtract)
                    else:
                        for h in range(2):
                            pa = slice(64 * h, 64 * h + 64)
                            nc.vector.tensor_tensor(
                                out=split_out[h][pa, sl],
                                in0=src_t[m][pa, sl], in1=bpc[pa, :],
                                op=mybir.AluOpType.subtract)

            # ---- attention (two head-pair streams interleaved) -----------
            ln_apply(qt, 0, 0)
            ln_apply(kt, 0, 2, split_out=kth[0])
            ln_apply(qt, 1, 1)
            ln_apply(kt, 1, 3, split_out=kth[1])

            def joint_finish(qb, otsbs):
                # one batched reciprocal for the 4 softmax-sum rows of this
                # query block (both streams x both heads, rows 32-spaced so
                # the PE broadcast matmuls are row-block aligned), then a
                # per-stream PE broadcast + multiply.
                rrec = rbp.tile([128, 512], FP32, tag="rb")
                nc.vector.reciprocal(out=rrec, in_=rcq[qb])
                rbf = rcpp.tile([128, 512], BF16, tag="rcb")
                nc.vector.tensor_copy(out=rbf, in_=rrec)
                for m in range(2):
                    nb = psu.tile([128, 512], FP32, tag="u", name="nb")
                    nc.tensor.matmul(nb, lhsT=selh_sb[:, m, :], rhs=rbf,
                                     start=True, stop=True)
                    nc.vector.tensor_tensor(
                        out=otb[m][:, qb * 512:(qb + 1) * 512],
                        in0=otsbs[m], in1=nb, op=mybir.AluOpType.mult)

            def emit_wo(t16):
                for e2 in range(2):
                    pso = psu.tile([128, 512], FP32, tag="u", name="pso")
                    for mm in range(2):
                        nc.tensor.matmul(
                            pso,
                            lhsT=otb[mm][:, t16 * 128:(t16 + 1) * 128],
                            rhs=wo_sb[:, mm, e2 * 512:(e2 + 1) * 512],
                            start=(mm == 0), stop=(mm == 1))
                    osb = outp.tile([128, 512], BF16, tag="osb")
                    nc.vector.tensor_copy(out=osb, in_=pso)
                    nc.sync.dma_start(
                        out=out_e.ap()[t16 * 128:(t16 + 1) * 128,
                                       e2 * 512:(e2 + 1) * 512],
                        in_=osb)

            def attn_stream(m):
                for qb in range(4):
                    otps = [psu1.tile([128, 512], FP32, tag="u1",
                                      name=f"otp{m}{h_}") for h_ in range(2)]
                    nkb = 4 * qb + 4
                    exq = []
                    for kb in range(nkb):
                        st = psu.tile([128, 1024], FP32, tag="u", name="st")
                        for h in range(2):
                            nc.tensor.matmul(
                                st[:, h * 512:(h + 1) * 512],
                                lhsT=kth[m][h][:, kb * 128:(kb + 1) * 128],
                                rhs=qt[m][:, qb * 512:(qb + 1) * 512],
                                start=True, stop=True)
                        ex = expp.tile([128, 1024], BF16, tag="exp")
                        nc.scalar.activation(
                            out=ex, in_=st,
                            func=mybir.ActivationFunctionType.Exp)
                        d = kb - 4 * qb
                        if d >= 0:  # diagonal block: causal 0/1 mask
                            # early diagonal tiles (pipeline slack) go to the
                            # idle GPSIMD engine; the flush-critical last two
                            # stay on the faster DVE
                            eng = nc.gpsimd if d < 2 else nc.vector
                            eng.tensor_tensor(
                                out=ex, in0=ex, in1=masks_sb[:, d, :],
                                op=mybir.AluOpType.mult)
                        exq.append((ex, kb))
                        if len(exq) > 3:
                            exp_, kb_ = exq.pop(0)
                            for h in range(2):
                                nc.tensor.matmul(
                                    otps[h][0:S + 1, :],
                                    lhsT=vhat[:, kb_, 2 * m + h, :],
                                    rhs=exp_[:, h * 512:(h + 1) * 512],
                                    start=(kb_ == 0), stop=False)
                        yield None
                    while exq:
                        exp_, kb_ = exq.pop(0)
                        for h in range(2):
                            nc.tensor.matmul(
                                otps[h][0:S + 1, :],
                                lhsT=vhat[:, kb_, 2 * m + h, :],
                                rhs=exp_[:, h * 512:(h + 1) * 512],
                                start=(kb_ == 0), stop=(kb_ == nkb - 1))
                    # evict O^T + sums rows off PSUM (DVE); the joint
                    # normalize happens in the driver once both streams
                    # finish this query block.
                    otsb = otsbp.tile([128, 512], FP32, tag="otsb",
                                      name=f"otsb{m}")
                    for h in range(2):
                        p = 64 * m + 32 * h
                        nc.vector.tensor_copy(out=rcq[qb][p:p + 1, :],
                                              in_=otps[h][S:S + 1, :])
                        nc.vector.tensor_copy(out=otsb[64 * h:64 * h + 64, :],
                                              in_=otps[h][0:S, :])
                    yield (qb, otsb)

            # per-qb tiles holding the 4 softmax-sum rows (2 streams x 2
            # heads): (m,h)'s row at partition 64m+32h. Unused partitions
            # are preset to 1.0 so the batched reciprocal stays finite
            # (their selector rows are zero, so they contribute nothing).
            rcq = [rcpp.tile([128, 512], FP32, tag="rc", name=f"rcq{q}")
                   for q in range(4)]
            for q in range(4):
                nc.vector.memset(rcq[q], 1.0)
            g0, g1 = attn_stream(0), attn_stream(1)
            done = [False, False]
            otsb_q = {}
            finished = [0, 0]

            def step(gi, g):
                if done[gi]:
                    return
                try:
                    r = next(g)
                    if isinstance(r, tuple):
                        qb, otsb = r
                        otsb_q[(gi, qb)] = otsb
                        finished[gi] = qb + 1
                        if (1 - gi, qb) in otsb_q:
                            joint_finish(qb, [otsb_q[(0, qb)],
                                              otsb_q[(1, qb)]])
                except StopIteration:
                    done[gi] = True

            for _ in range(2):  # stagger the streams by 2 kb-steps
                step(0, g0)
            emitted = 0
            while not (done[0] and done[1]):
                step(0, g0)
                step(1, g1)
                while emitted < 4 * min(finished):
                    emit_wo(emitted)
                    emitted += 1
            while emitted < 16:
                emit_wo(emitted)
                emitted += 1
    return nc




_NC_CACHE = None


def _get_nc():
    global _NC_CACHE
    if _NC_CACHE is None:
        _NC_CACHE = _build_bass()
    return _NC_CACHE


# ---------------------------------------------------------------------------
# Host wrapper
# ---------------------------------------------------------------------------

def _make_masks():
    # mask[p, d_idx, f] = 1.0 if p + d <= f else 0, d = 128*d_idx
    p = np.arange(128)[:, None, None]
    dd = (np.arange(4) * 128)[None, :, None]
    f = np.arange(512)[None, None, :]
    m = ((p + dd) <= f).astype(BF)           # [128, 4, 512]
    return np.concatenate([m, m], axis=2)    # [128, 4, 1024] (2 head halves)


def kernel(queries, keys, values, Wq, Wk, Wv, Wo, bo, q_ln_w, q_ln_b,
           k_ln_w, k_ln_b):
    from concourse.bass_utils import run_bass_kernel_spmd

    nc = _get_nc()

    masks = _make_masks()
    eye2 = np.zeros((128, 2), dtype=BF)
    eye2[0:64, 0] = 1
    eye2[64:128, 1] = 1
    # selector rows carry the folded ln weight (w * INV4); ln bias must be 0
    # (guaranteed by the input spec).
    wq_f = np.tile(np.asarray(q_ln_w, np.float32) * INV4, 2)   # [128]
    wk_f = np.tile(np.asarray(k_ln_w, np.float32) * INV4, 2)
    selrep = np.zeros((128, 4, 128), dtype=np.float32)
    for c, wrow in ((0, wq_f), (1, wq_f), (2, wk_f), (3, wk_f)):
        selrep[32 * c, c, 0:64] = wrow[0:64]
        selrep[32 * c + 1, c, 64:128] = wrow[64:128]
    selrep = selrep.astype(BF)
    # slice m: row 64m selects head 0 (otb partitions 0:64), row 64m+32
    # selects head 1 (partitions 64:128). All other rows stay zero.
    selh = np.zeros((128, 2, 128), dtype=BF)
    for m in range(2):
        selh[64 * m, m, 0:64] = 1
        selh[64 * m + 32, m, 64:128] = 1

    in_maps = []
    for core in range(8):
        b = core // 4
        cs = (core % 4) * EPC
        sl = slice(cs, cs + EPC)

        def parr(a, o, dt):
            # [o*128, f] -> [128, o, f] partition-contiguous layout
            a = np.asarray(a, np.float32)
            return np.ascontiguousarray(
                a.reshape(o, 128, a.shape[1]).transpose(1, 0, 2)).astype(dt)
        in_maps.append({
            "xtq": parr(np.asarray(queries[b], np.float32).T, 8, F8),
            "xtk": parr(np.asarray(keys[b], np.float32).T, 8, F8),
            "xtv": parr(np.asarray(values[b], np.float32).T, 8, BF),
            "wq": parr(np.asarray(Wq, np.float32)[:, sl], 8, F8),
            "wk": parr(np.asarray(Wk, np.float32)[:, sl], 8, F8),
            "wv": parr(np.asarray(Wv, np.float32)[:, sl], 8, BF),
            "wo": parr(np.asarray(Wo, np.float32)[sl, :], 2, BF),
            "masks": masks,
            "eye2": eye2,
            "selrep": selrep,
            "selh": selh,
        })

    kernel._last_in_maps = in_maps
    res = run_bass_kernel_spmd(nc, in_maps, core_ids=list(range(8)))
    outs = [res.results[i]["out"].astype(np.float32) for i in range(8)]
    bo32 = np.asarray(bo, np.float32)
    full = np.stack([
        outs[0] + outs[1] + outs[2] + outs[3] + bo32,
        outs[4] + outs[5] + outs[6] + outs[7] + bo32,
    ]).astype(np.float32)
    return full


# revision 43
# speedup vs baseline: 1.2654x; 1.0731x over previous
"""Multi-head attention with QK-LayerNorm on 8 TRN2 NeuronCores.

Shapes: B=2, T=2048, E=1024, H=16 heads, S=64 head dim.
Sharding: core c handles batch c//4 and the 4 heads [ (c%4)*4, (c%4)*4+4 ).
Each core computes a partial output (its heads' contribution through Wo);
the host sums the 4 partials per batch and adds bo.

Device-side layout (scores/PV/V/Wo matmuls bf16, Q/K projections fp8e4m3
DoubleRow, f32 PSUM accumulation; host pre-transposes activations and
pre-packs every DRAM tensor into partition-contiguous [128, o, f] layout
so DMAs are single descriptors):
  QT/KT   [s(64)*2heads = 128p, T]  transposed, 2 heads row-packed per tile
  V       [t_k 128p, head, s+1]     extra ones-column -> softmax row sums
  scores  S^T [t_k 128p, t_q 512]   strictly-above-causal blocks skipped
LayerNorm over s (the partition axis of QT) uses matmul statistics
(block-diagonal ones lhsT), f32 row math, then PE "selector" matmuls that
broadcast the per-(head,t) scale/shift rows into PSUM. The ln weight
(w*INV4) is folded into the selector matrix values so the apply is two
tensor_tensor ops (ln bias must be zero, which the input spec guarantees).
Softmax needs no max-subtraction: LN bounds logits to |q.k| <= ~2, so
exp() is taken directly off the scores PSUM (bf16 out), the causal mask
is a 0/1 multiply on diagonal blocks only, and row sums come free from
the ones-column of V. The two head-pair streams are interleaved with the
PV matmul pipelined 3 tiles behind exp so ACT latency never stalls PE.
During attention ACT does exp only: PSUM evictions go to GPSIMD/DVE, the
softmax denominator uses the fast DVE reciprocal, and the Wo matmuls +
output DMA are interleaved per finished 512-query block instead of
running as a serial tail.
"""

import json

import numpy as np
import ml_dtypes

import concourse.bass as bass
import concourse.bass2jax as bass2jax
import concourse.bass_utils as bass_utils
import concourse.tile as tile
from concourse import mybir

B, T, E, H, S = 2, 2048, 1024, 16, 64
HPC = 4            # heads per core
EPC = HPC * S      # feature cols per core = 256
LN_EPS = 1e-5
INV4 = float(E) ** -0.25
FP32 = mybir.dt.float32
BF16 = mybir.dt.bfloat16
FP8 = mybir.dt.float8e4
BF = ml_dtypes.bfloat16
F8 = ml_dtypes.float8_e4m3

# GPSIMD cannot touch PSUM on this toolchain, so PSUM->SBUF evictions go
# to DVE; GPSIMD instead takes half the (SBUF-only) causal mask multiplies.
ATTN_COPY_ENGINE = "vector"

# ---------------------------------------------------------------------------
# Compile hook: this toolchain's walrus accepts at most ONE semaphore wait per
# TPB instruction. Tile attaches several. Split extras into standalone
# EventSemaphore (wait-only) instructions on the same engine.
# ---------------------------------------------------------------------------
_TPB_ENGINES = ("Pool", "Activation", "PE", "DVE", "SP")


def _split_multiwaits(bir_json: bytes) -> bytes:
    d = json.loads(bir_json)
    n_split = 0
    for fn in d.get("functions", []):
        for blk in fn.get("blocks", []):
            insts = blk.get("instructions", [])
            out = []
            for inst in insts:
                si = inst.get("sync_info")
                waits = (si or {}).get("on_wait") or []
                if si and len(waits) > 1 and inst.get("engine") in _TPB_ENGINES:
                    for i, w in enumerate(waits[:-1]):
                        out.append({
                            "debug": inst.get("debug", 0),
                            "engine": inst["engine"],
                            "ins": [],
                            "name": f"{inst['name']}-ws{i}",
                            "opcode": "EventSemaphore",
                            "outs": [],
                            "sync_info": {"on_update": [], "on_wait": [w]},
                        })
                        n_split += 1
                    si["on_wait"] = [waits[-1]]
                out.append(inst)
            blk["instructions"] = out
    return json.dumps(d).encode()


_orig_compile_bir_kernel = bass_utils.compile_bir_kernel


def _patched_compile_bir_kernel(bir_json, tmpdir, neff_name="file.neff"):
    return _orig_compile_bir_kernel(_split_multiwaits(bir_json), tmpdir, neff_name)



bass_utils.compile_bir_kernel = _patched_compile_bir_kernel
bass2jax.compile_bir_kernel = _patched_compile_bir_kernel


def _patched_drain_and_barrier(self, tick_clock, wait_clock):
    # Same as TileContext._drain_and_barrier but the drain's waits are emitted
    # as single-wait instructions (walrus limit).
    gc = tick_clock.global_clock
    ticks = eval(str(gc).replace("VectorClock(", "").rstrip(")"))
    sems = wait_clock.sems.allocated()
    for proc_idx, sem in sems.items():
        t = ticks[proc_idx]
        if t > 0:
            mult = 16 if proc_idx >= 11 else 1
            self.nc.sync.wait_ge(sem, t * mult)
    self.nc.sync.drain()
    self.nc.all_engine_barrier()
    assert self.sems is not None
    popped = self.nc._tile_sem_poison_stack.pop()
    assert popped is self._sem_poison
    self.nc.clear_and_free_semaphores(list(self.sems.allocated().values()))
    self.nc.all_engine_barrier()


tile.TileContext._drain_and_barrier = _patched_drain_and_barrier


# ---------------------------------------------------------------------------
# Device kernel (identical program on all 8 cores)
# ---------------------------------------------------------------------------


def _act_raw(nc, out, in_, func):
    # nc.scalar.activation refuses Reciprocal (accuracy); our tolerance is
    # 2e-2 so the LUT version is fine. Emit InstActivation directly.
    eng = nc.scalar
    inputs = [eng.lower_ap(in_)]
    for arg in (0.0, 1.0, 0.0):  # bias, scale, alpha
        inputs.append(mybir.ImmediateValue(dtype=mybir.dt.float32, value=arg))
    return eng.add_instruction(
        mybir.InstActivation(
            name=nc.get_next_instruction_name(),
            func=func,
            ins=inputs,
            outs=[eng.lower_ap(out)],
        )
    )


def _build_bass():
    nc = bass.Bass()
    xtq_e = nc.dram_tensor("xtq", [128, 8, T], FP8, kind="ExternalInput")
    xtk_e = nc.dram_tensor("xtk", [128, 8, T], FP8, kind="ExternalInput")
    xtv_e = nc.dram_tensor("xtv", [128, 8, T], BF16, kind="ExternalInput")
    wq_e = nc.dram_tensor("wq", [128, 8, EPC], FP8, kind="ExternalInput")
    wk_e = nc.dram_tensor("wk", [128, 8, EPC], FP8, kind="ExternalInput")
    wv_e = nc.dram_tensor("wv", [128, 8, EPC], BF16, kind="ExternalInput")
    wo_e = nc.dram_tensor("wo", [128, 2, E], BF16, kind="ExternalInput")
    masks_e = nc.dram_tensor("masks", [128, 4, 1024], BF16, kind="ExternalInput")
    eye_e = nc.dram_tensor("eye2", [128, 2], BF16, kind="ExternalInput")
    selrep_e = nc.dram_tensor("selrep", [128, 4, 128], BF16, kind="ExternalInput")
    selh_e = nc.dram_tensor("selh", [128, 2, 128], BF16, kind="ExternalInput")
    out_e = nc.dram_tensor("out", [T, E], BF16, kind="ExternalOutput")

    xtq, xtk, xtv = xtq_e.ap(), xtk_e.ap(), xtv_e.ap()
    wq_a, wk_a, wv_a, wo_a = wq_e.ap(), wk_e.ap(), wv_e.ap(), wo_e.ap()
    DR = mybir.MatmulPerfMode.DoubleRow

    with tile.TileContext(nc) as tc:
        with tc.tile_pool(name="singles", bufs=1) as singles, \
             tc.tile_pool(name="xstream", bufs=4) as xstream, \
             tc.tile_pool(name="work", bufs=1) as work, \
             tc.tile_pool(name="rows", bufs=1) as rows, \
             tc.tile_pool(name="expp", bufs=10) as expp, \
             tc.tile_pool(name="outp", bufs=4) as outp, \
             tc.tile_pool(name="otsbp", bufs=2) as otsbp, \
             tc.tile_pool(name="rcp", bufs=6) as rcpp, \
             tc.tile_pool(name="rbp", bufs=2) as rbp, \
             tc.tile_pool(name="psu", bufs=2, space="PSUM") as psu, \
             tc.tile_pool(name="psu1", bufs=4, space="PSUM") as psu1:

            # ---- resident constants (issue order = DMA priority) ---------
            wq_sb = singles.tile([128, 8, EPC], FP8)
            wk_sb = singles.tile([128, 8, EPC], FP8)
            eye_sb = singles.tile([128, 2], BF16)
            nc.scalar.dma_start(out=eye_sb, in_=eye_e.ap())
            selrep_sb = singles.tile([128, 4, 128], BF16)
            nc.scalar.dma_start(out=selrep_sb, in_=selrep_e.ap())
            selh_sb = singles.tile([128, 2, 128], BF16)
            nc.scalar.dma_start(out=selh_sb, in_=selh_e.ap())
            xtv_sb = singles.tile([128, 8, T], BF16)
            wv_sb = singles.tile([128, 8, EPC], BF16)
            masks_sb = singles.tile([128, 4, 1024], BF16)
            wo_sb = singles.tile([128, 2, E], BF16)

            qt = [singles.tile([128, T], BF16, tag=f"qt{m}", name=f"qt{m}") for m in range(2)]
            kt = [singles.tile([128, T], BF16, tag=f"kt{m}", name=f"kt{m}") for m in range(2)]
            # per-head K tiles, zero-padded in the other head's rows so the
            # score matmuls contract over the full 128 partitions (K=128
            # matmuls run ~1.8x faster per column than K=64 on this HW)
            kth = [[singles.tile([128, T], BF16, tag=f"kth{m}{h}",
                                 name=f"kth{m}{h}") for h in range(2)]
                   for m in range(2)]
            vhat = singles.tile([128, 16, HPC, S + 1], BF16)
            otb = [singles.tile([128, T], BF16, tag=f"otb{m}", name=f"otb{m}") for m in range(2)]
            nc.vector.memset(vhat[:, :, :, S:S + 1], 1.0)
            for m in range(2):
                nc.vector.memset(kth[m][0][64:128, :], 0.0)
                nc.vector.memset(kth[m][1][0:64, :], 0.0)

            # ---- Q/K projections (fp8 DoubleRow) + LN stats --------------
            # stat rows live at partitions {32c, 32c+1}; the rest are preset
            # finite so the K=128-padded selector matmuls stay NaN-free
            sums_t = rows.tile([128, T], FP32)
            sumsq_t = rows.tile([128, T], FP32)
            nc.vector.memset(sums_t, 1.0)
            nc.vector.memset(sumsq_t, 1.0)

            def ln_stats(src_t, m, c):
                sq = work.tile([128, T], BF16, tag="sq")
                nc.vector.tensor_tensor(out=sq, in0=src_t[m], in1=src_t[m],
                                        op=mybir.AluOpType.mult)
                for n in range(4):
                    sl = slice(n * 512, (n + 1) * 512)
                    ps_s = psu1.tile([128, 512], FP32, tag="u1", name="st_s")
                    ps_q = psu1.tile([128, 512], FP32, tag="u1", name="st_q")
                    nc.tensor.matmul(ps_s[0:2, :], lhsT=eye_sb, rhs=src_t[m][:, sl],
                                     start=True, stop=True)
                    nc.tensor.matmul(ps_q[0:2, :], lhsT=eye_sb, rhs=sq[:, sl],
                                     start=True, stop=True)
                    if n % 2 == 0:
                        nc.scalar.activation(out=sums_t[32 * c:32 * c + 2, sl],
                                             in_=ps_s[0:2, :],
                                             func=mybir.ActivationFunctionType.Copy)
                        nc.scalar.activation(out=sumsq_t[32 * c:32 * c + 2, sl],
                                             in_=ps_q[0:2, :],
                                             func=mybir.ActivationFunctionType.Copy)
                    else:
                        nc.vector.tensor_copy(out=sums_t[32 * c:32 * c + 2, sl],
                                              in_=ps_s[0:2, :])
                        nc.vector.tensor_copy(out=sumsq_t[32 * c:32 * c + 2, sl],
                                              in_=ps_q[0:2, :])

            for qk_i, (x_ap, w_sb, w_a, dst) in enumerate(
                    ((xtq, wq_sb, wq_a, qt), (xtk, wk_sb, wk_a, kt))):
                if qk_i == 1:
                    nc.sync.dma_start(out=wk_sb, in_=wk_a)
                xcs = {}
                for m in range(2):
                    if qk_i == 1 and m == 1:
                        nc.sync.dma_start(out=wv_sb, in_=wv_a)
                        nc.sync.dma_start(out=masks_sb, in_=masks_e.ap())
                        nc.sync.dma_start(out=wo_sb, in_=wo_a)
                    pss = [psu.tile([128, 1024], FP32, tag="u", name=f"pss{j}")
                           for j in range(2)]
                    for kp in range(4):
                        if m == 0:
                            if qk_i == 0:
                                nc.sync.dma_start(out=wq_sb[:, 2 * kp:2 * kp + 2, :],
                                                  in_=wq_a[:, 2 * kp:2 * kp + 2, :])
                            xc = xstream.tile([128, 2, T], FP8, tag="xchunk",
                                              name=f"xc{kp}")
                            nc.sync.dma_start(out=xc, in_=x_ap[:, 2 * kp:2 * kp + 2, :])
                            if qk_i == 1:
                                nc.sync.dma_start(out=xtv_sb[:, 2 * kp:2 * kp + 2, :],
                                                  in_=xtv[:, 2 * kp:2 * kp + 2, :])
                            xcs[kp] = xc
                        xc = xcs[kp]
                        for n in range(4):
                            nc.tensor.matmul(
                                pss[n // 2][:, (n % 2) * 512:(n % 2) * 512 + 512],
                                lhsT=w_sb[:, 2 * kp:2 * kp + 2, m * 128:(m + 1) * 128],
                                rhs=xc[:, :, n * 512:(n + 1) * 512],
                                start=(kp == 0), stop=(kp == 3),
                                perf_mode=DR)
                    for j in range(2):
                        nc.vector.tensor_copy(
                            out=dst[m][:, j * 1024:(j + 1) * 1024], in_=pss[j])
                    ln_stats(dst, m, 2 * qk_i + m)

            # ---- LN row math (overlaps V projection) --------------------
            # mu = sums/S ; var = sumsq/S - mu^2 ; rstd = 1/sqrt(var+eps)
            # a-row = rstd ; c-row = mu*rstd  (w*INV4 folded into selrep)
            eps_col = singles.tile([128, 1], FP32)
            nc.vector.memset(eps_col, LN_EPS)
            tmp = rows.tile([128, T], FP32)
            nc.vector.scalar_tensor_tensor(
                out=tmp, in0=sums_t, scalar=1.0 / S, in1=sums_t,
                op0=mybir.AluOpType.mult, op1=mybir.AluOpType.mult)
            nc.vector.tensor_tensor(out=sumsq_t, in0=sumsq_t, in1=tmp,
                                    op=mybir.AluOpType.subtract)
            nc.scalar.activation(out=sumsq_t, in_=sumsq_t,
                                 func=mybir.ActivationFunctionType.Sqrt,
                                 bias=eps_col, scale=1.0 / S)
            _act_raw(nc, sumsq_t, sumsq_t,
                     mybir.ActivationFunctionType.Reciprocal)             # rstd
            c_bfrows = rows.tile([128, T], BF16)
            a_bfrows = rows.tile([128, T], BF16)
            nc.vector.scalar_tensor_tensor(
                out=c_bfrows, in0=sums_t, scalar=1.0 / S, in1=sumsq_t,
                op0=mybir.AluOpType.mult, op1=mybir.AluOpType.mult)       # mu*rstd
            nc.vector.tensor_copy(out=a_bfrows, in_=sumsq_t)

            # ---- V projection (natural layout + ones column) -------------
            for t16 in range(16):
                psv = psu.tile([128, 1024], FP32, tag="u", name="psv")
                for e8 in range(8):
                    nc.tensor.matmul(
                        psv[:, 0:EPC], lhsT=xtv_sb[:, e8, t16 * 128:(t16 + 1) * 128],
                        rhs=wv_sb[:, e8, :], start=(e8 == 0), stop=(e8 == 7))
                nc.scalar.activation(
                    out=vhat[:, t16, :, 0:S],
                    in_=psv[:, 0:EPC].rearrange("p (h s) -> p h s", h=HPC),
                    func=mybir.ActivationFunctionType.Copy)

            # ---- LN apply via PE row-broadcast --------------------------
            # bpa = (w*INV4)[p] * a_row(f), bpc = (w*INV4)[p] * c_row(f);
            # selrep carries the folded weights so the apply is 2 DVE ops.
            # K padded to the full 128 partitions (unused selector rows are
            # zero, the row tiles are finite everywhere).
            def ln_apply(src_t, m, c, split_out=None):
                sel = selrep_sb[:, c, :]
                for ch in range(4):
                    sl = slice(ch * 512, (ch + 1) * 512)
                    bpa = psu1.tile([128, 512], FP32, tag="u1", name="bpa")
                    bpc = psu1.tile([128, 512], FP32, tag="u1", name="bpc")
                    nc.tensor.matmul(bpa, lhsT=sel, rhs=a_bfrows[:, sl],
                                     start=True, stop=True)
                    nc.tensor.matmul(bpc, lhsT=sel, rhs=c_bfrows[:, sl],
                                     start=True, stop=True)
                    nc.vector.tensor_tensor(out=src_t[m][:, sl], in0=src_t[m][:, sl],
                                            in1=bpa,
                                            op=mybir.AluOpType.mult)
                    if split_out is None:
                        nc.vector.tensor_tensor(out=src_t[m][:, sl],
                                                in0=src_t[m][:, sl], in1=bpc,
                                                op=mybir.AluOpType.subtract)
                    else:
                        for h in range(2):
                            pa = slice(64 * h, 64 * h + 64)
                            nc.vector.tensor_tensor(
                                out=split_out[h][pa, sl],
                                in0=src_t[m][pa, sl], in1=bpc[pa, :],
                                op=mybir.AluOpType.subtract)

            # ---- attention (two head-pair streams interleaved) -----------
            ln_apply(qt, 0, 0)
            ln_apply(kt, 0, 2, split_out=kth[0])
            ln_apply(qt, 1, 1)
            ln_apply(kt, 1, 3, split_out=kth[1])

            def joint_finish(qb, otsbs):
                # one batched reciprocal for the 4 softmax-sum rows of this
                # query block (both streams x both heads, rows 32-spaced so
                # the PE broadcast matmuls are row-block aligned), then a
                # per-stream PE broadcast + multiply.
                rrec = rbp.tile([128, 512], FP32, tag="rb")
                nc.vector.reciprocal(out=rrec, in_=rcq[qb])
                rbf = rcpp.tile([128, 512], BF16, tag="rcb")
                nc.vector.tensor_copy(out=rbf, in_=rrec)
                for m in range(2):
                    nb = psu.tile([128, 512], FP32, tag="u", name="nb")
                    nc.tensor.matmul(nb, lhsT=selh_sb[:, m, :], rhs=rbf,
                                     start=True, stop=True)
                    nc.vector.tensor_tensor(
                        out=otb[m][:, qb * 512:(qb + 1) * 512],
                        in0=otsbs[m], in1=nb, op=mybir.AluOpType.mult)

            def emit_wo(t16):
                for e2 in range(2):
                    pso = psu.tile([128, 512], FP32, tag="u", name="pso")
                    for mm in range(2):
                        nc.tensor.matmul(
                            pso,
                            lhsT=otb[mm][:, t16 * 128:(t16 + 1) * 128],
                            rhs=wo_sb[:, mm, e2 * 512:(e2 + 1) * 512],
                            start=(mm == 0), stop=(mm == 1))
                    osb = outp.tile([128, 512], BF16, tag="osb")
                    nc.vector.tensor_copy(out=osb, in_=pso)
                    nc.sync.dma_start(
                        out=out_e.ap()[t16 * 128:(t16 + 1) * 128,
                                       e2 * 512:(e2 + 1) * 512],
                        in_=osb)

            def attn_stream(m):
                # continuous pipeline: the PV drain (and the PSUM eviction it
                # ends with) trails the score/exp stream across query-block
                # boundaries, so the PE never stalls at a block edge.
                exq = []  # (ex, kb, qb, otps)

                def drain_one():
                    ex_, kb_, qb_, otps_ = exq.pop(0)
                    last = kb_ == 4 * qb_ + 3
                    for h in range(2):
                        nc.tensor.matmul(
                            otps_[h][0:S + 1, :],
                            lhsT=vhat[:, kb_, 2 * m + h, :],
                            rhs=ex_[:, h * 512:(h + 1) * 512],
                            start=(kb_ == 0), stop=last)
                    if not last:
                        return None
                    # query block finished: evict O^T + sum rows off PSUM
                    otsb = otsbp.tile([128, 512], FP32, tag="otsb",
                                      name=f"otsb{m}")
                    for h in range(2):
                        p = 64 * m + 32 * h
                        nc.vector.tensor_copy(out=rcq[qb_][p:p + 1, :],
                                              in_=otps_[h][S:S + 1, :])
                        nc.vector.tensor_copy(out=otsb[64 * h:64 * h + 64, :],
                                              in_=otps_[h][0:S, :])
                    return (qb_, otsb)

                for qb in range(4):
                    otps = [psu1.tile([128, 512], FP32, tag="u1",
                                      name=f"otp{m}{h_}") for h_ in range(2)]
                    for kb in range(4 * qb + 4):
                        st = psu.tile([128, 1024], FP32, tag="u", name="st")
                        for h in range(2):
                            nc.tensor.matmul(
                                st[:, h * 512:(h + 1) * 512],
                                lhsT=kth[m][h][:, kb * 128:(kb + 1) * 128],
                                rhs=qt[m][:, qb * 512:(qb + 1) * 512],
                                start=True, stop=True)
                        ex = expp.tile([128, 1024], BF16, tag="exp")
                        nc.scalar.activation(
                            out=ex, in_=st,
                            func=mybir.ActivationFunctionType.Exp)
                        d = kb - 4 * qb
                        if d >= 0:  # diagonal block: causal 0/1 mask
                            # early diagonal tiles (pipeline slack) go to the
                            # idle GPSIMD engine; the flush-critical last two
                            # stay on the faster DVE
                            eng = nc.gpsimd if d < 2 else nc.vector
                            eng.tensor_tensor(
                                out=ex, in0=ex, in1=masks_sb[:, d, :],
                                op=mybir.AluOpType.mult)
                        exq.append((ex, kb, qb, otps))
                        yield drain_one() if len(exq) > 3 else None
                while exq:
                    yield drain_one()

            # per-qb tiles holding the 4 softmax-sum rows (2 streams x 2
            # heads): (m,h)'s row at partition 64m+32h. Unused partitions
            # are preset to 1.0 so the batched reciprocal stays finite
            # (their selector rows are zero, so they contribute nothing).
            rcq = [rcpp.tile([128, 512], FP32, tag="rc", name=f"rcq{q}")
                   for q in range(4)]
            for q in range(4):
                nc.vector.memset(rcq[q], 1.0)
            g0, g1 = attn_stream(0), attn_stream(1)
            done = [False, False]
            otsb_q = {}
            finished = [0, 0]

            def step(gi, g):
                if done[gi]:
                    return
                try:
                    r = next(g)
                    if isinstance(r, tuple):
                        qb, otsb = r
                        otsb_q[(gi, qb)] = otsb
                        finished[gi] = qb + 1
                        if (1 - gi, qb) in otsb_q:
                            joint_finish(qb, [otsb_q[(0, qb)],
                                              otsb_q[(1, qb)]])
                except StopIteration:
                    done[gi] = True

            for _ in range(2):  # stagger the streams by 2 kb-steps
                step(0, g0)
            emitted = 0
            while not (done[0] and done[1]):
                step(0, g0)
                step(1, g1)
                while emitted < 4 * min(finished):
                    emit_wo(emitted)
                    emitted += 1
            while emitted < 16:
                emit_wo(emitted)
                emitted += 1
    return nc




_NC_CACHE = None


def _get_nc():
    global _NC_CACHE
    if _NC_CACHE is None:
        _NC_CACHE = _build_bass()
    return _NC_CACHE


# ---------------------------------------------------------------------------
# Host wrapper
# ---------------------------------------------------------------------------

def _make_masks():
    # mask[p, d_idx, f] = 1.0 if p + d <= f else 0, d = 128*d_idx
    p = np.arange(128)[:, None, None]
    dd = (np.arange(4) * 128)[None, :, None]
    f = np.arange(512)[None, None, :]
    m = ((p + dd) <= f).astype(BF)           # [128, 4, 512]
    return np.concatenate([m, m], axis=2)    # [128, 4, 1024] (2 head halves)


def kernel(queries, keys, values, Wq, Wk, Wv, Wo, bo, q_ln_w, q_ln_b,
           k_ln_w, k_ln_b):
    from concourse.bass_utils import run_bass_kernel_spmd

    nc = _get_nc()

    masks = _make_masks()
    eye2 = np.zeros((128, 2), dtype=BF)
    eye2[0:64, 0] = 1
    eye2[64:128, 1] = 1
    # selector rows carry the folded ln weight (w * INV4); ln bias must be 0
    # (guaranteed by the input spec).
    wq_f = np.tile(np.asarray(q_ln_w, np.float32) * INV4, 2)   # [128]
    wk_f = np.tile(np.asarray(k_ln_w, np.float32) * INV4, 2)
    selrep = np.zeros((128, 4, 128), dtype=np.float32)
    for c, wrow in ((0, wq_f), (1, wq_f), (2, wk_f), (3, wk_f)):
        selrep[32 * c, c, 0:64] = wrow[0:64]
        selrep[32 * c + 1, c, 64:128] = wrow[64:128]
    selrep = selrep.astype(BF)
    # slice m: row 64m selects head 0 (otb partitions 0:64), row 64m+32
    # selects head 1 (partitions 64:128). All other rows stay zero.
    selh = np.zeros((128, 2, 128), dtype=BF)
    for m in range(2):
        selh[64 * m, m, 0:64] = 1
        selh[64 * m + 32, m, 64:128] = 1

    in_maps = []
    for core in range(8):
        b = core // 4
        cs = (core % 4) * EPC
        sl = slice(cs, cs + EPC)

        def parr(a, o, dt):
            # [o*128, f] -> [128, o, f] partition-contiguous layout
            a = np.asarray(a, np.float32)
            return np.ascontiguousarray(
                a.reshape(o, 128, a.shape[1]).transpose(1, 0, 2)).astype(dt)
        in_maps.append({
            "xtq": parr(np.asarray(queries[b], np.float32).T, 8, F8),
            "xtk": parr(np.asarray(keys[b], np.float32).T, 8, F8),
            "xtv": parr(np.asarray(values[b], np.float32).T, 8, BF),
            "wq": parr(np.asarray(Wq, np.float32)[:, sl], 8, F8),
            "wk": parr(np.asarray(Wk, np.float32)[:, sl], 8, F8),
            "wv": parr(np.asarray(Wv, np.float32)[:, sl], 8, BF),
            "wo": parr(np.asarray(Wo, np.float32)[sl, :], 2, BF),
            "masks": masks,
            "eye2": eye2,
            "selrep": selrep,
            "selh": selh,
        })

    kernel._last_in_maps = in_maps
    res = run_bass_kernel_spmd(nc, in_maps, core_ids=list(range(8)))
    outs = [res.results[i]["out"].astype(np.float32) for i in range(8)]
    bo32 = np.asarray(bo, np.float32)
    full = np.stack([
        outs[0] + outs[1] + outs[2] + outs[3] + bo32,
        outs[4] + outs[5] + outs[6] + outs[7] + bo32,
    ]).astype(np.float32)
    return full
